# revision 1
# baseline (speedup 1.0000x reference)
"""Trainium2 Bass kernel for nn_MatchingLayer (BiMPM-style matching layer).

Full inputs: con_p/con_h (32, 64, 400) fp32 + 8 perspective weights (20, 200).
Returns (mv_p, mv_h), each (32, 64, 160) fp32.

Sharding: pure data parallel over batch — each of the 8 NeuronCores gets 4
examples; the weights are replicated.

Numerics: interior tensors are bf16 (PE runs 1 cyc/row vs 4 for fp32; DMA
broadcast volume halves); accumulations (PSUM), norms and the final cosine
tails stay fp32.  The overall output error budget is rel_l2 < 2e-2; measured
~1.7e-3 with this mix.

Structure: a 3-stage software pipeline over the 8 per-core (example,
direction) units — load(k+2) | phaseA(k+1) prep+staging+broadcasts |
phaseS(k) all max-scans + pairwise | phaseT(k-1) streams+tails+outputs.
The dominant cost is the fused mul+max DVE scan (cosine-weighted
max-attentive matching, ~1 cycle per element per lane).  Scan lanes are
packed: h-rows 0..128 at full 128 lanes per side, rows 128..192 of BOTH
sides of a unit in one 128-lane group (PE shift-matmuls at the legal
0/32/64 partition bases move data in/out), and rows 192..200 via
page-dimension scans fed straight from SBUF.  PSUM is budgeted to exactly
8 banks with phase-local ring tags so the pipeline stages decouple.

Key algebraic simplifications vs the reference (exactness notes):
 - cosine similarity cos(v1*w, v2*w) is invariant under any POSITIVE scalar
   rescaling of v2 rows.  The reference's attentive-mean branch divides
   att-sums by row sums through _dsmall (which divides by a POSITIVE number
   in all cases), and its attentive branches carry positive row scales
   1/||p_row||, 1/||h_row||.  All of those cancel in the final cosine, so we
   drop them and only keep the scale factors that sit INSIDE a sum or a max.
 - att[p,q] = G[p,q] * rn1[p] * rn2[q] with G = p_fw @ h_fw^T (the eps guard
   in the reference never triggers for this data -- denominators are O(100)
   vs eps=1e-8; validated in test.py).
"""

import numpy as np
import sys

for _p in ("/opt/trn_rl_repo",):
    if _p not in sys.path:
        sys.path.insert(0, _p)

import concourse.bass as bass
import concourse.bacc as bacc
import concourse.tile as tile
from concourse import mybir
from concourse.masks import make_identity
from concourse import bass_utils

# ---------------------------------------------------------------------------
# Custom DVE op: fused multiply + per-page-resetting max-scan.
# With a stride-0 innermost output AP this acts as a grouped max-reduce of
# in0*in1 in a single VectorE pass.
# ---------------------------------------------------------------------------
import concourse.dve_spec as dve_spec
from concourse.dve_spec import (
    Spec,
    Src0,
    Src1,
    AluOp,
    Scan,
    lower,
    _Stage,
    _scan_init,
)
from concourse.dve_ops import DveOp, OPS, CUSTOM_DVE_SPECS
from concourse.dve_uop import DveOpSpec

# identity registry of scans that reset at SUB_DIM_DONE
_SEG_SCAN_IDS: set[int] = set()

_orig_scan_overrides = dve_spec._scan_overrides


def _patched_scan_overrides(scans, node_stage):
    seed, step = _orig_scan_overrides(scans, node_stage)
    for scan in scans:
        if id(scan) in _SEG_SCAN_IDS:
            d = node_stage[scan]
            step[d] = _Stage(scan.op, _scan_init(scan), scan.expr)
    return seed, step


dve_spec._scan_overrides = _patched_scan_overrides


def _make_op():
    expr = Src0 * Src1
    sc = Scan(AluOp.MAX, expr)
    _SEG_SCAN_IDS.add(id(sc))
    body = sc

    def _ref(in0, in1, s0, s1, imm2):
        prod = (in0.astype(np.float32) * in1).astype(np.float32)
        return np.maximum.accumulate(prod, axis=-1)

    spec = Spec(body=body, reference=_ref)

    shas = {}
    for ver in ("v3", "v4"):
        try:
            uops = lower(spec, ver=ver)
            tmp = DveOpSpec(name="MUL_MAXSCAN_SEG", opcode=0, uops=uops, rd1_en=True)
            shas[ver] = tmp.sha(ver)
        except Exception:
            pass

    for _existing in OPS:
        if _existing.name == "MUL_MAXSCAN_SEG":
            return _existing
    op = DveOp(
        "MUL_MAXSCAN_SEG",
        spec,
        subdim=True,
        uops_sha=shas,
    )
    OPS.append(op)
    CUSTOM_DVE_SPECS[op.name] = op.spec
    import concourse.dve_ops as _dops
    _dops._SUB_OPCODE_FOR_NAME[op.name] = (
        _dops._CUSTOM_DVE_ROW_BASE + len(OPS) - 1
    )
    assert _dops._SUB_OPCODE_FOR_NAME[op.name] < 0x20
    return op


MUL_MAXSCAN_SEG = _make_op()


def mul_maxscan_seg(nc, out_ap, in0_ap, in1_ap):
    """out = per-page (innermost-dim-resetting) running max of in0*in1.
    All APs [P, S, N]."""
    return nc.vector._custom_dve(
        MUL_MAXSCAN_SEG, out=out_ap, in0=in0_ap, in1=in1_ap
    )


F32 = mybir.dt.float32
BF16 = mybir.dt.bfloat16
AF = mybir.ActivationFunctionType

B, S, H, L = 32, 64, 200, 20
NCORES = 8
BPC = B // NCORES  # examples per core
CHUNKS = ((0, 128), (128, 72))  # h-dim split across partitions
SLOT_FULL, SLOT_MAX, SLOT_MEAN, SLOT_AMAX = 0, 1, 2, 3



def _build_core_program():
    nc = bacc.Bacc("TRN2", target_bir_lowering=False, debug=False)

    cp = nc.dram_tensor("cp", [BPC, S, 2 * H], F32, kind="ExternalInput").ap()
    ch = nc.dram_tensor("ch", [BPC, S, 2 * H], F32, kind="ExternalInput").ap()
    w_drams = [
        nc.dram_tensor(f"w{i+1}", [L, H], F32, kind="ExternalInput").ap()
        for i in range(8)
    ]
    mvp = nc.dram_tensor("mvp", [BPC, S, 8 * L], F32, kind="ExternalOutput").ap()
    mvh = nc.dram_tensor("mvh", [BPC, S, 8 * L], F32, kind="ExternalOutput").ap()

    with tile.TileContext(nc) as tc:
        _trace_kernel(nc, tc, cp, ch, w_drams, mvp, mvh)

    nc.compile()
    return nc


def _trace_kernel(nc, tc, cp, ch, w_drams, mvp, mvh):
    from contextlib import ExitStack

    ctx = ExitStack()
    with ctx:
        # ---------------- pools ----------------
        # PSUM budget (8 banks): psA bf16 x2, mmA f32 x2, mmB f32 x1,
        # blk x1, comb x2.
        const_pool = ctx.enter_context(tc.tile_pool(name="const", bufs=1))
        w2t_pool = ctx.enter_context(tc.tile_pool(name="w2t", bufs=1))
        in_pool = ctx.enter_context(tc.tile_pool(name="inp", bufs=4))
        t_pool = ctx.enter_context(tc.tile_pool(name="tlay", bufs=3))
        sc_pool = ctx.enter_context(tc.tile_pool(name="scal", bufs=4))
        big_pool = ctx.enter_context(tc.tile_pool(name="bigsb", bufs=2))
        ab_pool = ctx.enter_context(tc.tile_pool(name="abpool", bufs=3))
        stage_pool = ctx.enter_context(tc.tile_pool(name="stage", bufs=4))
        lg_pool = ctx.enter_context(tc.tile_pool(name="lgpool", bufs=3))
        ps_A = ctx.enter_context(tc.tile_pool(name="ps_A", bufs=2, space="PSUM"))
        ps_mmA = ctx.enter_context(tc.tile_pool(name="ps_mmA", bufs=2, space="PSUM"))
        ps_mmB = ctx.enter_context(tc.tile_pool(name="ps_mmB", bufs=1, space="PSUM"))
        ps_blk = ctx.enter_context(tc.tile_pool(name="ps_blk", bufs=1, space="PSUM"))

        ps_nA = ctx.enter_context(tc.tile_pool(name="ps_nA", bufs=2, space="PSUM"))
        dram_pool = ctx.enter_context(tc.tile_pool(name="drst", bufs=4, space="DRAM"))

        def bcast_rows(dst_ap, dram_ap, nparts, eng=None):
            """dst[(m, *free)] <- dram[*free] for all m (stride-0 partition dim)."""
            src = bass.AP(
                tensor=dram_ap.tensor,
                offset=dram_ap.offset,
                ap=[[0, nparts]] + [list(x) for x in dram_ap.ap],
            )
            (eng or nc.sync).dma_start(dst_ap, src)

        # ---------------- constants ----------------
        identb = const_pool.tile([64, 64], BF16, tag="identb")
        make_identity(nc, identb[:])
        identf = const_pool.tile([64, 64], F32, tag="identf")
        make_identity(nc, identf[:])
        ones_col = const_pool.tile([128, 1], BF16, tag="ones")
        nc.vector.memset(ones_col[:], 1.0)
        ident2 = const_pool.tile([128, 64], BF16, tag="ident2")
        nc.vector.memset(ident2[:], 0.0)
        make_identity(nc, ident2[64:128, :])
        oneh = const_pool.tile([128, 4, 32], BF16, tag="oneh")
        nc.vector.memset(oneh[:], 0.0)
        for _s in range(4):
            make_identity(nc, oneh[64 : 64 + 8, _s, 8 * _s : 8 * _s + 8])

        # W2T[i]: squared weight, transposed: chunks (128,20),(72,20), bf16
        w2t = [None] * 8

        def build_w2t(i):
            w_nat = in_pool.tile([L, H], F32, tag="w_nat", name=f"w_nat{i}")
            nc.sync.dma_start(w_nat[:], w_drams[i])
            w2_nat = in_pool.tile([L, H], BF16, tag="w2_nat", name=f"w2n{i}")
            nc.gpsimd.tensor_tensor(
                w2_nat[:], w_nat[:], w_nat[:], op=mybir.AluOpType.mult
            )
            wt = w2t_pool.tile([128, 2, L], BF16, tag=f"w2t{i}", name=f"w2t{i}")
            tp = ps_A.tile([128, 16, 64], BF16, tag="psA", name=f"w2tp{i}")
            for c, (c0, cs) in enumerate(CHUNKS):
                nc.tensor.transpose(
                    tp[:cs, c, :L], w2_nat[:, c0 : c0 + cs], identb[:L, :L]
                )
            nc.scalar.copy(wt[:, 0, :], tp[:, 0, :L])
            nc.scalar.copy(wt[:72, 1, :], tp[:72, 1, :L])
            w2t[i] = wt

        # ---------------- interleaved phase A / phase B -----------------
        persist = ctx.enter_context(tc.tile_pool(name="persist", bufs=8))
        rb_pool = ctx.enter_context(tc.tile_pool(name="rbpool", bufs=3))

        st = {}
        mvhmax_tiles = {}
        stgs = {}
        for b in range(BPC):
            stgs[b] = (
                stage_pool.tile([S, 8, L], F32, tag="stg_p", name=f"stgp{b}"),
                stage_pool.tile([S, 8, L], F32, tag="stg_h", name=f"stgh{b}"),
            )

        loads = {}

        def loadA(b, d):
            off = d * H
            P_nat = in_pool.tile([S, H], F32, tag="P_nat", name=f"P_nat{b}{d}")
            H_nat = in_pool.tile([S, H], F32, tag="H_nat", name=f"H_nat{b}{d}")
            nc.sync.dma_start(P_nat[:], cp[b, :, off : off + H])
            nc.sync.dma_start(H_nat[:], ch[b, :, off : off + H])
            loads[(b, d)] = (P_nat, H_nat)

        def phaseA(b, d):
            w_pair = w2t[2 + d]
            u = 2 * b + d
            # per-unit PSUM homes (ring of 2 units)
            psA = ps_A.tile([128, 16, 64], BF16, tag="psA", name=f"psA{u}")
            mmA = ps_mmA.tile([128, 2, H], F32, tag="mmA", name=f"mmA{u}")

            P_nat, H_nat = loads.pop((b, d))
            P_natb = persist.tile([S, H], BF16, tag="P_natb")
            H_natb = persist.tile([S, H], BF16, tag="H_natb")
            nc.scalar.copy(P_natb[:], P_nat[:])
            nc.scalar.copy(H_natb[:], H_nat[:])

            # ---- transposed layouts + squares (bf16) ----
            # psA slots: 0,1 P chunks; 2,3 H chunks; 4 P rows 128..192 at
            # base 64 (for the packed chunk1b group)
            PT = persist.tile([128, 2, S], BF16, tag="PT")
            HT = persist.tile([128, 2, S], BF16, tag="HT")
            PT2 = persist.tile([128, 2, S], BF16, tag="PT2")
            HT2 = persist.tile([128, 2, S], BF16, tag="HT2")
            for c, (c0, cs) in enumerate(CHUNKS):
                nc.tensor.transpose(
                    psA[:cs, c, :S], P_natb[:, c0 : c0 + cs], identb[:]
                )
                nc.tensor.transpose(
                    psA[:cs, 2 + c, :S], H_natb[:, c0 : c0 + cs], identb[:]
                )
            nc.tensor.transpose(
                psA[S : 2 * S, 4, :S], P_natb[:, 128:192], identb[:]
            )
            nc.scalar.copy(PT[:], psA[:, 0:2, :])
            nc.scalar.copy(HT[:], psA[:, 2:4, :])
            nc.scalar.square(PT2[:], psA[:, 0:2, :])
            nc.scalar.square(HT2[:], psA[:, 2:4, :])
            VT1u = lg_pool.tile([128, S], BF16, tag="VT1u", name=f"VT1u{u}")
            nc.scalar.copy(VT1u[0:S], psA[0:S, 3, :])
            nc.scalar.copy(VT1u[S : 2 * S], psA[S : 2 * S, 4, :S])

            # ---- row norms -> rn (1/||row||), fp32 ----
            nsq = mmA[:S, :, 128:129].rearrange("s c one -> s (c one)")
            for c, (c0, cs) in enumerate(CHUNKS):
                nc.tensor.matmul(
                    nsq[:, 0:1], PT2[:cs, c, :], ones_col[:cs, :],
                    start=(c == 0), stop=(c == 1),
                )
            for c, (c0, cs) in enumerate(CHUNKS):
                nc.tensor.matmul(
                    nsq[:, 1:2], HT2[:cs, c, :], ones_col[:cs, :],
                    start=(c == 0), stop=(c == 1),
                )
            nrm = sc_pool.tile([S, 2], F32, tag="nrm")
            nc.scalar.sqrt(nrm[:], nsq[:])
            rn = sc_pool.tile([S, 2], F32, tag="rn")
            nc.vector.reciprocal_approx_fast(rn[:], nrm[:])
            rn1 = rn[:, 0:1]
            rn2 = rn[:, 1:2]

            # ---- G (p,q) and GT (q,p) ----
            gps = mmA[:S, :, 0:S]
            for c, (c0, cs) in enumerate(CHUNKS):
                nc.tensor.matmul(
                    gps[:, 0, :], PT[:cs, c, :], HT[:cs, c, :],
                    start=(c == 0), stop=(c == 1),
                )
            for c, (c0, cs) in enumerate(CHUNKS):
                nc.tensor.matmul(
                    gps[:, 1, :], HT[:cs, c, :], PT[:cs, c, :],
                    start=(c == 0), stop=(c == 1),
                )
            G2 = big_pool.tile([S, S], BF16, tag="G2")
            GsT = big_pool.tile([S, S], BF16, tag="GsT")
            nc.scalar.mul(G2[:], gps[:, 0, :], rn1)
            nc.scalar.mul(GsT[:], gps[:, 1, :], rn2)

            # transposes: Gs = GsT^T, G2T = G2^T -> stage to DRAM (bf16)
            nc.tensor.transpose(psA[:S, 5, :S], GsT[:], identb[:])
            nc.tensor.transpose(psA[:S, 6, :S], G2[:], identb[:])
            GsG2T = big_pool.tile([S, 2, S], BF16, tag="GsG2T")
            nc.scalar.copy(GsG2T[:], psA[:S, 5:7, :])
            att_dram = dram_pool.tile([2, S, S], BF16, tag="att_d")
            nc.sync.dma_start(att_dram[:].transpose([1, 0, 2]), GsG2T[:])

            # broadcasts for the scans (issued now: no HOL blocking later)
            attP = ab_pool.tile([128, S, S], BF16, tag="attP")
            attH = ab_pool.tile([128, S, S], BF16, tag="attH")
            bcast_rows(attP[:, 0 : S // 2, :], att_dram[0][0 : S // 2], 128)
            bcast_rows(attP[:, S // 2 :, :], att_dram[0][S // 2 :], 128)
            bcast_rows(attH[:, 0 : S // 2, :], att_dram[1][0 : S // 2], 128)
            bcast_rows(attH[:, S // 2 :, :], att_dram[1][S // 2 :], 128)
            att1u = lg_pool.tile([128, S, S], BF16, tag="att1u", name=f"att1u{u}")
            bcast_rows(att1u[0:S], att_dram[0], S)
            bcast_rows(att1u[S : 2 * S], att_dram[1], S)
            # chunk2 (rows 192..200): stage the 8 v2T rows of each side and
            # replicate them across 64 lanes for the page-dim scans
            vt2_dram = dram_pool.tile([2, 8, S], BF16, tag="vt2_d")
            nc.sync.dma_start(vt2_dram[0], HT[S : S + 8, 1, :])
            nc.sync.dma_start(vt2_dram[1], PT[S : S + 8, 1, :])
            rep2 = lg_pool.tile([S, 2, 8, S], BF16, tag="rep2", name=f"rep2{u}")
            bcast_rows(rep2[:], vt2_dram[:], S)

            # ---- attentive-mean sums ----
            msps = ps_mmB.tile([S, 2, H], F32, tag="mmB", name=f"mmB{u}")
            nc.tensor.matmul(msps[:, 0, :], GsT[:], H_natb[:], start=True, stop=True)
            nc.tensor.matmul(msps[:, 1, :], G2[:], P_natb[:], start=True, stop=True)
            MS = big_pool.tile([S, 2, H], BF16, tag="MS")
            nc.scalar.copy(MS[:], msps[:])
            MS_T = persist.tile([128, 4, S], BF16, tag="MS_T")
            for c, (c0, cs) in enumerate(CHUNKS):
                nc.tensor.transpose(
                    psA[:cs, 8 + c, :S], MS[:, 0, c0 : c0 + cs], identb[:]
                )
                nc.tensor.transpose(
                    psA[:cs, 10 + c, :S], MS[:, 1, c0 : c0 + cs], identb[:]
                )
            nc.scalar.copy(MS_T[:], psA[:, 8:12, :])

            # ---- pairwise norms rna/rnb ----
            nabps = mmA[:L, :, 136:200]
            for c, (c0, cs) in enumerate(CHUNKS):
                nc.tensor.matmul(nabps[:, 0, :], w_pair[:cs, c, :], PT2[:cs, c, :],
                                 start=(c == 0), stop=(c == 1))
            for c, (c0, cs) in enumerate(CHUNKS):
                nc.tensor.matmul(nabps[:, 1, :], w_pair[:cs, c, :], HT2[:cs, c, :],
                                 start=(c == 0), stop=(c == 1))
            nab_s = sc_pool.tile([L, 2, S], F32, tag="nab_s")
            nc.scalar.sqrt(nab_s[:], nabps[:])
            rnab = sc_pool.tile([L, 2, S], F32, tag="rnab")
            nc.vector.reciprocal_approx_fast(rnab[:], nab_s[:])
            rnab_b = sc_pool.tile([L, 2, S], BF16, tag="rnab_b")
            nc.scalar.copy(rnab_b[:], rnab[:])
            nc.tensor.transpose(psA[:S, 7, :L], rnab_b[:, 0, :], identb[:L, :L])
            tprf = mmA[:, 0, 136:156]
            nc.tensor.matmul(
                tprf[S : 2 * S], rnab_b[:, 1, :], identb[:L, :L],
                tile_position=(0, 64), start=True, stop=True,
            )
            rnT = persist.tile([128, 2, L], BF16, tag="rnT")
            nc.scalar.copy(rnT[:S, 0, :], psA[:S, 7, :L])
            nc.scalar.copy(rnT[S : 2 * S, 1, :], tprf[S : 2 * S])
            rnab_dram = dram_pool.tile([L, 2, S], BF16, tag="rnab_d")
            nc.sync.dma_start(rnab_dram[:], rnab_b[:])
            RAB = rb_pool.tile([128, L, S], BF16, tag="RAB")
            bcast_rows(RAB[0:S], rnab_dram[:, 1, :], S)
            bcast_rows(RAB[S : 2 * S], rnab_dram[:, 0, :], S)

            SH = big_pool.tile([128, 2, L, S], BF16, tag="SH")
            for c, (c0, cs) in enumerate(CHUNKS):
                ht_b = HT[:cs, c, :][:, None, :].broadcast_to([cs, L, S])
                w_b = w_pair[:cs, c, :][:, :, None].broadcast_to([cs, L, S])
                nc.gpsimd.tensor_tensor(
                    SH[:cs, c], ht_b, w_b, op=mybir.AluOpType.mult
                )
            st[(b, d)] = dict(
                P_natb=P_natb, H_natb=H_natb, PT=PT, HT=HT, PT2=PT2, HT2=HT2,
                MS_T=MS_T, rnT=rnT, attP=attP, attH=attH, RAB=RAB,
                att_dram=att_dram, VT1u=VT1u, att1u=att1u, rep2=rep2,
                GsG2T=GsG2T, SH=SH, psA=psA, mmA=mmA,
            )


        def phaseS(b, d):
            v = st[(b, d)]
            PT, HT = v["PT"], v["HT"]
            attP, attH = v["attP"], v["attH"]
            w_pair = w2t[2 + d]
            u = 2 * b + d

            # ---- max-attentive chunk0 scans ----
            MhT = t_pool.tile([128, 2, S], BF16, tag="MhT", name=f"MhT{u}")
            MpT = t_pool.tile([128, 2, S], BF16, tag="MpT", name=f"MpT{u}")
            PG = S // 2
            for (VT, attB, MT) in ((HT, attP, MhT), (PT, attH, MpT)):
                for g in range(2):
                    sl = slice(g * PG, (g + 1) * PG)
                    vt_b = VT[:, 0, :][:, None, :].broadcast_to([128, PG, S])
                    out_v = MT[:, 0, sl][:, :, None].broadcast_to([128, PG, S])
                    mul_maxscan_seg(nc, out_v, vt_b, attB[:, sl, :])
            v["MhT"], v["MpT"] = MhT, MpT
            # chunk1b scan: rows 128..192, both sides
            MT1u = lg_pool.tile([128, S], BF16, tag="MT1u", name=f"MT1u{u}")
            in0 = v["VT1u"][:, None, :].broadcast_to([128, S, S])
            out_v = MT1u[:, :, None].broadcast_to([128, S, S])
            mul_maxscan_seg(nc, out_v, in0, v["att1u"][:])
            v["MT1u"] = MT1u
            # chunk2 page-dim scans: rows 192..200, lanes = the non-scanned
            # sequence index, pages = the 8 h rows
            GsG2T = v["GsG2T"]
            rep2 = v["rep2"]
            MT2s = lg_pool.tile([S, 2, 8], BF16, tag="MT2s", name=f"MT2s{u}")
            for s in range(2):
                in0 = GsG2T[:, s, :][:, None, :].broadcast_to([S, 8, S])
                out_v = MT2s[:, s, :][:, :, None].broadcast_to([S, 8, S])
                mul_maxscan_seg(nc, out_v, in0, rep2[:, s])
            v["MT2s"] = MT2s

            # ---- pairwise (maxpool) matching ----
            SH = v["SH"]
            RAB = v["RAB"]
            pmxC = sc_pool.tile([128, L], F32, tag="pmxC", name=f"pmxC{u}")
            # asymmetric pass sizes: front-load the work; the comb PSUM ring
            # (depth 2) stalls the later passes, so keeping them tiny lets
            # their supply chains fit inside the DVE window (-12us vs 5/5/5/5)
            l0 = 0
            for lh, LH in enumerate((8, 7, 4, 1)):
                lsl = slice(l0, l0 + LH)
                l0 += LH
                comb = ps_nA.tile([128, 8, S], F32, tag="nA",
                                  name=f"comb{u}_{lh}")[:, 0:LH, :]
                nA = comb[0:S]
                nA_flat = nA.rearrange("p l q -> p (l q)")
                for c, (c0, cs) in enumerate(CHUNKS):
                    sh_flat = SH[:cs, c, lsl, :].rearrange("h l q -> h (l q)")
                    nc.tensor.matmul(
                        nA_flat[:],
                        PT[:cs, c, :],
                        sh_flat[:],
                        start=(c == 0),
                        stop=(c == 1),
                    )
                nAs = big_pool.tile([S, 8, S], F32, tag="nAs",
                                    name=f"nAs{u}_{lh}")[:, 0:LH, :]
                # two half-copies so the first transposes start sooner
                if LH > 2:
                    nc.scalar.copy(nAs[:, 0:2, :], nA[:, 0:2, :])
                    nc.scalar.copy(nAs[:, 2:LH, :], nA[:, 2:LH, :])
                else:
                    nc.scalar.copy(nAs[:], nA[:])
                for l in range(LH):
                    nc.tensor.matmul(
                        comb[S : 2 * S, l, :],
                        nAs[:, l, :],
                        identf[:],
                        tile_position=(0, 64),
                    )
                out_v = pmxC[:, lsl][:, :, None].broadcast_to([128, LH, S])
                mul_maxscan_seg(nc, out_v, comb[:], RAB[:, lsl, :])
            v["pmxC"] = pmxC

        def phaseT(b, d):
            stg_p, stg_h = stgs[b]
            v = st[(b, d)]
            PT, HT, PT2, HT2 = v["PT"], v["HT"], v["PT2"], v["HT2"]
            MS_hT = v["MS_T"][:, 0:2, :]
            MS_pT = v["MS_T"][:, 2:4, :]
            rnaT = v["rnT"][:, 0, :]
            MhT, MpT = v["MhT"], v["MpT"]
            pmxC = v["pmxC"]
            w_full = w2t[0 + d]
            w_mean = w2t[4 + d]
            w_amax = w2t[6 + d]
            so = 4 * d
            u = 2 * b + d

            # rows 128..192 from the per-unit packed group
            MT1u = v["MT1u"]
            nc.scalar.copy(MhT[0:S, 1, :], MT1u[0:S])
            shq = v["mmA"][:, 1, 64:128]
            nc.tensor.matmul(shq[0:S], ident2[S : 2 * S, :],
                             MT1u[S : 2 * S], start=True, stop=True)
            nc.scalar.copy(MpT[0:S, 1, :], shq[0:S])
            # rows 192..200 from the page-dim scans: one transpose per side
            MT2s = v["MT2s"]
            psAu = v["psA"]
            for side, MT in ((0, MhT), (1, MpT)):
                nc.tensor.transpose(
                    psAu[S : S + 8, 13 + side, :S], MT2s[:, side, :], identb[:],
                )
                nc.scalar.copy(MT[S : S + 8, 1, :], psAu[S : S + 8, 13 + side, :S])

            mvhmax = sc_pool.tile([128, L], F32, tag="mvhmax", name=f"mvhm{u}")
            nc.gpsimd.tensor_tensor(
                mvhmax[S : 2 * S], pmxC[S : 2 * S],
                v["rnT"][S : 2 * S, 1, :], op=mybir.AluOpType.mult,
            )
            mvhmax_tiles[(b, d)] = mvhmax

            # ---- mp_match cosine accumulators: one PSUM bank ----
            blk = ps_blk.tile([S, 6, 4, L], F32, tag="blk")
            DOTp, NAp, NBp = blk[:, 0], blk[:, 1], blk[:, 2]
            DOTh, NAh, NBh = blk[:, 3], blk[:, 4], blk[:, 5]

            def mm_to(dst_ap, streams):
                for c, (c0, cs) in enumerate(CHUNKS):
                    nc.tensor.matmul(
                        dst_ap, streams[0](c), streams[1](c),
                        start=(c == 0), stop=(c == 1),
                    )

            def mp_stream(DOT, NA, NB, slot, v1T2, zT, z2T, w):
                mm_to(DOT[:, slot, :], (lambda c: zT[: CHUNKS[c][1], c, :],
                                        lambda c: w[: CHUNKS[c][1], c, :]))
                mm_to(NA[:, slot, :], (lambda c: v1T2[: CHUNKS[c][1], c, :],
                                       lambda c: w[: CHUNKS[c][1], c, :]))
                mm_to(NB[:, slot, :], (lambda c: z2T[: CHUNKS[c][1], c, :],
                                       lambda c: w[: CHUNKS[c][1], c, :]))

            def mk_z(v1T, v2T, tag, split=False):
                zT = t_pool.tile([128, 2, S], BF16, tag=f"zT_{tag}")
                z2T = t_pool.tile([128, 2, S], BF16, tag=f"z2T_{tag}")
                if split:
                    # per-chunk ops so chunk-0 streams start before the
                    # chunk-1 repack lands
                    for c in range(2):
                        nc.gpsimd.tensor_tensor(
                            zT[:, c, :], v1T[:, c, :], v2T[:, c, :],
                            op=mybir.AluOpType.mult,
                        )
                        nc.scalar.square(z2T[:, c, :], v2T[:, c, :])
                else:
                    nc.gpsimd.tensor_tensor(
                        zT[:], v1T[:, :, :], v2T[:, :, :],
                        op=mybir.AluOpType.mult,
                    )
                    nc.scalar.square(z2T[:], v2T[:, :, :])
                return zT, z2T

            class _ColBcast:
                def __init__(self, col2):
                    self.col2 = col2

                def __getitem__(self, idx):
                    sl, c, _ = idx
                    return self.col2[sl, c : c + 1].broadcast_to(
                        [self.col2[sl].shape[0], S]
                    )

            def mp_full(DOT, NA, NB, v1T, v1T2, vT, w, tag, j):
                zT = t_pool.tile([128, 2, S], BF16, tag=f"zT_f{tag}")
                col2 = sc_pool.tile([128, 2], BF16, tag=f"col2_{tag}")
                colf = sc_pool.tile([128, 2], F32, tag=f"colf_{tag}")
                nc.scalar.copy(colf[:], vT[:, :, j])
                for c, (c0, cs) in enumerate(CHUNKS):
                    nc.scalar.mul(
                        zT[:cs, c, :], v1T[:cs, c, :], colf[:cs, c : c + 1]
                    )
                nc.scalar.square(col2[:], vT[:, :, j])
                mp_stream(DOT, NA, NB, SLOT_FULL, v1T2, zT, _ColBcast(col2), w)

            j = 63 if d == 0 else 0
            z, z2 = mk_z(PT, MhT, "pamax", split=True)
            mp_stream(DOTp, NAp, NBp, SLOT_AMAX, PT2, z, z2, w_amax)
            z, z2 = mk_z(HT, MpT, "hamax", split=True)
            mp_stream(DOTh, NAh, NBh, SLOT_AMAX, HT2, z, z2, w_amax)
            mp_full(DOTp, NAp, NBp, PT, PT2, HT, w_full, "p", j)
            mp_full(DOTh, NAh, NBh, HT, HT2, PT, w_full, "h", j)
            z, z2 = mk_z(PT, MS_hT, "pmean")
            mp_stream(DOTp, NAp, NBp, SLOT_MEAN, PT2, z, z2, w_mean)
            z, z2 = mk_z(HT, MS_pT, "hmean")
            mp_stream(DOTh, NAh, NBh, SLOT_MEAN, HT2, z, z2, w_mean)

            # ---- cosine tails: both sides batched per instruction ----
            nab2 = sc_pool.tile([S, 6, 4, L], F32, tag="nab2")
            nc.scalar.copy(nab2[:], blk[:])
            prod = sc_pool.tile([S, 8, L], F32, tag="prod")
            nc.gpsimd.tensor_tensor(
                prod[:].rearrange("s (a b) l -> s a b l", a=2),
                nab2[:, 1::3], nab2[:, 2::3], op=mybir.AluOpType.mult,
            )
            dd = sc_pool.tile([S, 8, L], F32, tag="dd")
            nc.scalar.sqrt(dd[:], prod[:])
            r = sc_pool.tile([S, 8, L], F32, tag="r")
            nc.vector.reciprocal_approx_fast(r[:], dd[:])
            for side, stg in ((0, stg_p), (1, stg_h)):
                nc.gpsimd.tensor_tensor(
                    stg[:, so : so + 4, :], nab2[:, 3 * side],
                    r[:, 4 * side : 4 * side + 4, :],
                    op=mybir.AluOpType.mult,
                )
            nc.gpsimd.tensor_tensor(
                stg_p[:, so + SLOT_MAX, :], pmxC[0:S], rnaT[0:S],
                op=mybir.AluOpType.mult,
            )

        def emit_outputs(b):
            stg_p, stg_h = stgs[b]
            nc.gpsimd.dma_start(mvp[b], stg_p[:].rearrange("s k l -> s (k l)"))
            # the over-p max rows are ready before the cosine tails; write the
            # rest of mvh around their columns so nothing serializes on them
            for d in range(2):
                so = 4 * d
                nc.gpsimd.dma_start(
                    mvh[b, :, (so + SLOT_MAX) * L : (so + SLOT_MAX + 1) * L],
                    mvhmax_tiles[(b, d)][S : 2 * S],
                )
            for c0, c1 in ((0, SLOT_MAX), (SLOT_MAX + 1, 4 + SLOT_MAX), (4 + SLOT_MAX + 1, 8)):
                nc.gpsimd.dma_start(
                    mvh[b, :, c0 * L : c1 * L],
                    stg_h[:, c0:c1, :].rearrange("s k l -> s (k l)"),
                )

        units = [(b, d) for b in range(BPC) for d in range(2)]
        NU = len(units)
        build_w2t(2)
        build_w2t(3)
        loadA(*units[0])
        loadA(*units[1])
        phaseA(*units[0])
        for i in (0, 1, 4, 5, 6, 7):
            build_w2t(i)
        # steady state: load(k+2) | A(k+1) | S(k) | T(k-1)
        for k in range(NU):
            if k + 2 < NU:
                loadA(*units[k + 2])  # depth-2 lookahead
            if k + 1 < NU:
                phaseA(*units[k + 1])
            phaseS(*units[k])
            if k >= 1:
                phaseT(*units[k - 1])
                if units[k - 1][1] == 1:
                    emit_outputs(units[k - 1][0])
        phaseT(*units[-1])
        emit_outputs(units[-1][0])


_CACHE = {}


def _get_program():
    if "nc" not in _CACHE:
        _CACHE["nc"] = _build_core_program()
    return _CACHE["nc"]


def _run(inputs, trace=False):
    nc = _get_program()
    con_p = np.ascontiguousarray(np.asarray(inputs["con_p"], dtype=np.float32))
    con_h = np.ascontiguousarray(np.asarray(inputs["con_h"], dtype=np.float32))
    in_maps = []
    for c in range(NCORES):
        m = {
            "cp": con_p[c * BPC : (c + 1) * BPC],
            "ch": con_h[c * BPC : (c + 1) * BPC],
        }
        for i in range(8):
            m[f"w{i+1}"] = np.ascontiguousarray(
                np.asarray(inputs[f"mp_w{i+1}"], dtype=np.float32)
            )
        in_maps.append(m)
    res = bass_utils.run_bass_kernel_spmd(
        nc, in_maps, core_ids=list(range(NCORES)), trace=trace
    )
    mv_p = np.concatenate([r["mvp"] for r in res.results], axis=0)
    mv_h = np.concatenate([r["mvh"] for r in res.results], axis=0)
    return (mv_p, mv_h), res


def kernel(**inputs):
    out, _ = _run(inputs, trace=False)
    return out


def predicted_ns():
    """Cost-model (TimelineSim) predicted on-device duration in ns."""
    try:
        from trails.perfetto import LazyPerfetto

        def _noop(self, *a, **k):
            return 0

        for m in (
            "enable_explicit_ordering",
            "reserve_process_order",
            "add_counter",
        ):
            if not hasattr(LazyPerfetto, m):
                setattr(LazyPerfetto, m, _noop)
        from concourse.timeline_sim import TimelineSim

        return float(TimelineSim(_get_program(), trace=False).simulate())
    except Exception:
        return None


def run_profiled(**inputs):
    """Run on hardware; return (outputs, exec_time_ns).

    NTFF hardware tracing is unavailable under this axon client (no
    antenv.axon_hooks), so exec time falls back to the cost-model
    prediction, which the optimization loop was driven by.
    """
    try:
        out, res = _run(inputs, trace=True)
        ns = res.exec_time_ns
    except ModuleNotFoundError:
        out, _ = _run(inputs, trace=False)
        ns = None
    if ns is None:
        ns = predicted_ns()
    return out, ns



# revision 42
# speedup vs baseline: 1.0325x; 1.0325x over previous
"""Trainium2 Bass kernel for nn_MatchingLayer (BiMPM-style matching layer).

Full inputs: con_p/con_h (32, 64, 400) fp32 + 8 perspective weights (20, 200).
Returns (mv_p, mv_h), each (32, 64, 160) fp32.

Sharding: pure data parallel over batch — each of the 8 NeuronCores gets 4
examples; the weights are replicated.

Numerics: interior tensors are bf16 (PE runs 1 cyc/row vs 4 for fp32; DMA
broadcast volume halves); accumulations (PSUM), norms and the final cosine
tails stay fp32.  The overall output error budget is rel_l2 < 2e-2; measured
~1.7e-3 with this mix.

Structure: a 3-stage software pipeline over the 8 per-core (example,
direction) units — load(k+2) | phaseA(k+1) prep+staging+broadcasts |
phaseS(k) all max-scans + pairwise | phaseT(k-1) streams+tails+outputs.
The dominant cost is the fused mul+max DVE scan (cosine-weighted
max-attentive matching, ~1 cycle per element per lane).  Scan lanes are
packed: h-rows 0..128 at full 128 lanes per side, rows 128..192 of BOTH
sides of a unit in one 128-lane group (PE shift-matmuls at the legal
0/32/64 partition bases move data in/out), and rows 192..200 via
page-dimension scans fed straight from SBUF.  PSUM is budgeted to exactly
8 banks with phase-local ring tags so the pipeline stages decouple.

Key algebraic simplifications vs the reference (exactness notes):
 - cosine similarity cos(v1*w, v2*w) is invariant under any POSITIVE scalar
   rescaling of v2 rows.  The reference's attentive-mean branch divides
   att-sums by row sums through _dsmall (which divides by a POSITIVE number
   in all cases), and its attentive branches carry positive row scales
   1/||p_row||, 1/||h_row||.  All of those cancel in the final cosine, so we
   drop them and only keep the scale factors that sit INSIDE a sum or a max.
 - att[p,q] = G[p,q] * rn1[p] * rn2[q] with G = p_fw @ h_fw^T (the eps guard
   in the reference never triggers for this data -- denominators are O(100)
   vs eps=1e-8; validated in test.py).
"""

import numpy as np
import sys

for _p in ("/opt/trn_rl_repo",):
    if _p not in sys.path:
        sys.path.insert(0, _p)

import concourse.bass as bass
import concourse.bacc as bacc
import concourse.tile as tile
from concourse import mybir
from concourse.masks import make_identity
from concourse import bass_utils

# ---------------------------------------------------------------------------
# Custom DVE op: fused multiply + per-page-resetting max-scan.
# With a stride-0 innermost output AP this acts as a grouped max-reduce of
# in0*in1 in a single VectorE pass.
# ---------------------------------------------------------------------------
import concourse.dve_spec as dve_spec
from concourse.dve_spec import (
    Spec,
    Src0,
    Src1,
    AluOp,
    Scan,
    lower,
    _Stage,
    _scan_init,
)
from concourse.dve_ops import DveOp, OPS, CUSTOM_DVE_SPECS
from concourse.dve_uop import DveOpSpec

# identity registry of scans that reset at SUB_DIM_DONE
_SEG_SCAN_IDS: set[int] = set()

_orig_scan_overrides = dve_spec._scan_overrides


def _patched_scan_overrides(scans, node_stage):
    seed, step = _orig_scan_overrides(scans, node_stage)
    for scan in scans:
        if id(scan) in _SEG_SCAN_IDS:
            d = node_stage[scan]
            step[d] = _Stage(scan.op, _scan_init(scan), scan.expr)
    return seed, step


dve_spec._scan_overrides = _patched_scan_overrides


def _make_op():
    expr = Src0 * Src1
    sc = Scan(AluOp.MAX, expr)
    _SEG_SCAN_IDS.add(id(sc))
    body = sc

    def _ref(in0, in1, s0, s1, imm2):
        prod = (in0.astype(np.float32) * in1).astype(np.float32)
        return np.maximum.accumulate(prod, axis=-1)

    spec = Spec(body=body, reference=_ref)

    shas = {}
    for ver in ("v3", "v4"):
        try:
            uops = lower(spec, ver=ver)
            tmp = DveOpSpec(name="MUL_MAXSCAN_SEG", opcode=0, uops=uops, rd1_en=True)
            shas[ver] = tmp.sha(ver)
        except Exception:
            pass

    for _existing in OPS:
        if _existing.name == "MUL_MAXSCAN_SEG":
            return _existing
    op = DveOp(
        "MUL_MAXSCAN_SEG",
        spec,
        subdim=True,
        uops_sha=shas,
    )
    OPS.append(op)
    CUSTOM_DVE_SPECS[op.name] = op.spec
    import concourse.dve_ops as _dops
    _dops._SUB_OPCODE_FOR_NAME[op.name] = (
        _dops._CUSTOM_DVE_ROW_BASE + len(OPS) - 1
    )
    assert _dops._SUB_OPCODE_FOR_NAME[op.name] < 0x20
    return op


MUL_MAXSCAN_SEG = _make_op()


def mul_maxscan_seg(nc, out_ap, in0_ap, in1_ap):
    """out = per-page (innermost-dim-resetting) running max of in0*in1.
    All APs [P, S, N]."""
    return nc.vector._custom_dve(
        MUL_MAXSCAN_SEG, out=out_ap, in0=in0_ap, in1=in1_ap
    )


F32 = mybir.dt.float32
BF16 = mybir.dt.bfloat16
AF = mybir.ActivationFunctionType


def rsqrt(nc, sc_pool, out_ap, in_ap, tag):
    """1/sqrt(x): Act sqrt followed by the fast DVE reciprocal.
    (Act's Rsqrt/Abs_reciprocal_sqrt tables are inaccurate — measured
    2e-1 rel error — so the two-op form stays.)"""
    shp = list(in_ap.shape)
    tmp = sc_pool.tile(shp, F32, tag=tag)
    nc.scalar.sqrt(tmp[:], in_ap)
    return nc.vector.reciprocal_approx_fast(out_ap, tmp[:])

B, S, H, L = 32, 64, 200, 20
NCORES = 8
BPC = B // NCORES  # examples per core
CHUNKS = ((0, 128), (128, 72))  # h-dim split across partitions
SLOT_FULL, SLOT_MAX, SLOT_MEAN, SLOT_AMAX = 0, 1, 2, 3



def _build_core_program():
    nc = bacc.Bacc("TRN2", target_bir_lowering=False, debug=False)

    cp = nc.dram_tensor("cp", [BPC, S, 2 * H], F32, kind="ExternalInput").ap()
    ch = nc.dram_tensor("ch", [BPC, S, 2 * H], F32, kind="ExternalInput").ap()
    w_drams = [
        nc.dram_tensor(f"w{i+1}", [L, H], F32, kind="ExternalInput").ap()
        for i in range(8)
    ]
    mvp = nc.dram_tensor("mvp", [BPC, S, 8 * L], F32, kind="ExternalOutput").ap()
    mvh = nc.dram_tensor("mvh", [BPC, S, 8 * L], F32, kind="ExternalOutput").ap()

    with tile.TileContext(nc) as tc:
        _trace_kernel(nc, tc, cp, ch, w_drams, mvp, mvh)

    nc.compile()
    return nc


def _trace_kernel(nc, tc, cp, ch, w_drams, mvp, mvh):
    from contextlib import ExitStack

    ctx = ExitStack()
    with ctx:
        # ---------------- pools ----------------
        # PSUM budget (8 banks): psA bf16 x2, mmA f32 x2, mmB f32 x1,
        # blk x1, comb x2.
        const_pool = ctx.enter_context(tc.tile_pool(name="const", bufs=1))
        w2t_pool = ctx.enter_context(tc.tile_pool(name="w2t", bufs=1))
        in_pool = ctx.enter_context(tc.tile_pool(name="inp", bufs=4))
        t_pool = ctx.enter_context(tc.tile_pool(name="tlay", bufs=3))
        sc_pool = ctx.enter_context(tc.tile_pool(name="scal", bufs=4))
        big_pool = ctx.enter_context(tc.tile_pool(name="bigsb", bufs=3))
        gs_pool = ctx.enter_context(tc.tile_pool(name="gspk", bufs=3))
        ab_pool = ctx.enter_context(tc.tile_pool(name="abpool", bufs=3))
        stage_pool = ctx.enter_context(tc.tile_pool(name="stage", bufs=4))
        lg_pool = ctx.enter_context(tc.tile_pool(name="lgpool", bufs=3))
        ps_A = ctx.enter_context(tc.tile_pool(name="ps_A", bufs=2, space="PSUM"))
        ps_mmA = ctx.enter_context(tc.tile_pool(name="ps_mmA", bufs=1, space="PSUM"))
        ps_mmB = ctx.enter_context(tc.tile_pool(name="ps_mmB", bufs=1, space="PSUM"))
        ps_T = ctx.enter_context(tc.tile_pool(name="ps_T", bufs=1, space="PSUM"))
        ps_blk = ctx.enter_context(tc.tile_pool(name="ps_blk", bufs=1, space="PSUM"))

        ps_nA = ctx.enter_context(tc.tile_pool(name="ps_nA", bufs=2, space="PSUM"))
        dram_pool = ctx.enter_context(tc.tile_pool(name="drst", bufs=4, space="DRAM"))

        def bcast_rows(dst_ap, dram_ap, nparts, eng=None):
            """dst[(m, *free)] <- dram[*free] for all m (stride-0 partition dim)."""
            src = bass.AP(
                tensor=dram_ap.tensor,
                offset=dram_ap.offset,
                ap=[[0, nparts]] + [list(x) for x in dram_ap.ap],
            )
            (eng or nc.sync).dma_start(dst_ap, src)

        def bcast_sides(dst_ap, dram_ap2, n, nrep, eng=None):
            """dst[(g*nrep + m), 0:n] <- dram2[g, 0:n] for g in (0,1), all m:
            one DMA that broadcasts side g of a [2, n] DRAM block to
            partitions g*nrep..(g+1)*nrep."""
            src = bass.AP(
                tensor=dram_ap2.tensor,
                offset=dram_ap2.offset,
                ap=[[n, 2], [0, nrep], [1, n]],
            )
            (eng or nc.sync).dma_start(dst_ap, src)

        # ---------------- constants ----------------
        identb = const_pool.tile([64, 64], BF16, tag="identb")
        make_identity(nc, identb[:])
        identf = const_pool.tile([64, 64], F32, tag="identf")
        make_identity(nc, identf[:])
        ones_col = const_pool.tile([128, 1], BF16, tag="ones")
        nc.vector.memset(ones_col[:], 1.0)
        ident2 = const_pool.tile([128, 64], BF16, tag="ident2")
        nc.vector.memset(ident2[:], 0.0)
        make_identity(nc, ident2[64:128, :])
        ident128 = const_pool.tile([128, 128], BF16, tag="ident128")
        make_identity(nc, ident128[:])
        oneh = const_pool.tile([128, 4, 32], BF16, tag="oneh")
        nc.vector.memset(oneh[:], 0.0)
        for _s in range(4):
            make_identity(nc, oneh[64 : 64 + 8, _s, 8 * _s : 8 * _s + 8])

        # W2T[i]: squared weight, transposed: chunks (128,20),(72,20), bf16
        w2t = [None] * 8

        def build_w2t(i):
            w_nat = in_pool.tile([L, H], F32, tag="w_nat", name=f"w_nat{i}")
            nc.sync.dma_start(w_nat[:], w_drams[i])
            w2_nat = in_pool.tile([L, H], BF16, tag="w2_nat", name=f"w2n{i}")
            nc.gpsimd.tensor_tensor(
                w2_nat[:], w_nat[:], w_nat[:], op=mybir.AluOpType.mult
            )
            wt = w2t_pool.tile([128, 2, L], BF16, tag=f"w2t{i}", name=f"w2t{i}")
            tp = ps_A.tile([128, 16, 64], BF16, tag="psA", name=f"w2tp{i}")
            for c, (c0, cs) in enumerate(CHUNKS):
                nc.tensor.transpose(
                    tp[:cs, c, :L], w2_nat[:, c0 : c0 + cs], identb[:L, :L]
                )
            nc.scalar.copy(wt[:, 0, :], tp[:, 0, :L])
            nc.scalar.copy(wt[:72, 1, :], tp[:72, 1, :L])
            w2t[i] = wt

        # ---------------- interleaved phase A / phase B -----------------
        persist = ctx.enter_context(tc.tile_pool(name="persist", bufs=8))
        rb_pool = ctx.enter_context(tc.tile_pool(name="rbpool", bufs=3))

        st = {}
        mvhmax_tiles = {}
        stgs = {}
        for b in range(BPC):
            stgs[b] = (
                stage_pool.tile([S, 8, L], F32, tag="stg_p", name=f"stgp{b}"),
                stage_pool.tile([S, 8, L], F32, tag="stg_h", name=f"stgh{b}"),
            )

        loads = {}

        def loadA(b, d):
            off = d * H
            P_nat = in_pool.tile([S, H], F32, tag="P_nat", name=f"P_nat{b}{d}")
            H_nat = in_pool.tile([S, H], F32, tag="H_nat", name=f"H_nat{b}{d}")
            nc.sync.dma_start(P_nat[:], cp[b, :, off : off + H])
            nc.sync.dma_start(H_nat[:], ch[b, :, off : off + H])
            loads[(b, d)] = (P_nat, H_nat)

        def phaseA(b, d):
            w_pair = w2t[2 + d]
            u = 2 * b + d
            # per-unit PSUM homes (ring of 2 units)
            psA = ps_A.tile([128, 16, 64], BF16, tag="psA", name=f"psA{u}")
            mmA = ps_mmA.tile([128, 2, H], F32, tag="mmA", name=f"mmA{u}")

            P_nat, H_nat = loads.pop((b, d))
            P_natb = persist.tile([S, H], BF16, tag="P_natb")
            H_natb = persist.tile([S, H], BF16, tag="H_natb")
            nc.scalar.copy(P_natb[:], P_nat[:])
            nc.scalar.copy(H_natb[:], H_nat[:])

            # ---- transposed layouts + squares (bf16) ----
            # psA slots: 0,1 P chunks; 2,3 H chunks; 4 P rows 128..192 at
            # base 64 (for the packed chunk1b group)
            PHT = persist.tile([128, 4, S], BF16, tag="PHT")
            PHT2 = persist.tile([128, 4, S], BF16, tag="PHT2")
            PT, HT = PHT[:, 0:2, :], PHT[:, 2:4, :]
            PT2, HT2 = PHT2[:, 0:2, :], PHT2[:, 2:4, :]
            for c, (c0, cs) in enumerate(CHUNKS):
                nc.tensor.transpose(
                    psA[:cs, c, :S], P_natb[:, c0 : c0 + cs], identb[:]
                )
                nc.tensor.transpose(
                    psA[:cs, 2 + c, :S], H_natb[:, c0 : c0 + cs], identb[:]
                )
            nc.tensor.transpose(
                psA[S : 2 * S, 4, :S], P_natb[:, 128:192], identb[:]
            )
            nc.scalar.copy(PHT[:], psA[:, 0:4, :])
            nc.scalar.square(PHT2[:], psA[:, 0:4, :])
            VT1u = lg_pool.tile([128, S], BF16, tag="VT1u", name=f"VT1u{u}")
            nc.scalar.copy(VT1u[0:S], psA[0:S, 3, :])
            nc.scalar.copy(VT1u[S : 2 * S], psA[S : 2 * S, 4, :S])

            # ---- row norms -> rn (1/||row||), fp32 ----
            nsq = mmA[:S, :, 128:129].rearrange("s c one -> s (c one)")
            for c, (c0, cs) in enumerate(CHUNKS):
                nc.tensor.matmul(
                    nsq[:, 0:1], PT2[:cs, c, :], ones_col[:cs, :],
                    start=(c == 0), stop=(c == 1),
                )
            for c, (c0, cs) in enumerate(CHUNKS):
                nc.tensor.matmul(
                    nsq[:, 1:2], HT2[:cs, c, :], ones_col[:cs, :],
                    start=(c == 0), stop=(c == 1),
                )
            rn = sc_pool.tile([S, 2], F32, tag="rn")
            rsqrt(nc, sc_pool, rn[:], nsq[:], "nrm")
            rn1 = rn[:, 0:1]
            rn2 = rn[:, 1:2]

            # ---- G (p,q) and GT (q,p) ----
            gps = mmA[:S, :, 0:S]
            for c, (c0, cs) in enumerate(CHUNKS):
                nc.tensor.matmul(
                    gps[:, 0, :], PT[:cs, c, :], HT[:cs, c, :],
                    start=(c == 0), stop=(c == 1),
                )
            for c, (c0, cs) in enumerate(CHUNKS):
                nc.tensor.matmul(
                    gps[:, 1, :], HT[:cs, c, :], PT[:cs, c, :],
                    start=(c == 0), stop=(c == 1),
                )
            G2 = big_pool.tile([S, S], BF16, tag="G2")
            GsT = big_pool.tile([S, S], BF16, tag="GsT")
            nc.scalar.mul(G2[:], gps[:, 0, :], rn1)
            nc.scalar.mul(GsT[:], gps[:, 1, :], rn2)

            # transposes: Gs = GsT^T at parts 0..64, G2T = G2^T at parts
            # 64..128 -> ONE [128, S] side-packed tile (clean staging DMA +
            # direct chunk2 scan input)
            nc.tensor.transpose(psA[:S, 5, :S], GsT[:], identb[:])
            nc.tensor.transpose(
                psA[S : 2 * S, 5, :S], G2[:], identb[:], tile_position=(0, 64)
            )
            GsPK = gs_pool.tile([128, S], BF16, tag="GsPK")
            nc.scalar.copy(GsPK[:], psA[:, 5, :])

            # ---- attentive-mean sums ----
            msps = ps_mmB.tile([S, 2, H], F32, tag="mmB", name=f"mmB{u}")
            nc.tensor.matmul(msps[:, 0, :], GsT[:], H_natb[:], start=True, stop=True)
            nc.tensor.matmul(msps[:, 1, :], G2[:], P_natb[:], start=True, stop=True)
            MS = big_pool.tile([S, 2, H], BF16, tag="MS")
            nc.scalar.copy(MS[:], msps[:])
            MS_T = persist.tile([128, 4, S], BF16, tag="MS_T")
            for c, (c0, cs) in enumerate(CHUNKS):
                nc.tensor.transpose(
                    psA[:cs, 8 + c, :S], MS[:, 0, c0 : c0 + cs], identb[:]
                )
                nc.tensor.transpose(
                    psA[:cs, 10 + c, :S], MS[:, 1, c0 : c0 + cs], identb[:]
                )
            nc.scalar.copy(MS_T[:], psA[:, 8:12, :])

            # ---- pairwise norms rna/rnb ----
            nabps = mmA[:L, :, 136:200]
            for c, (c0, cs) in enumerate(CHUNKS):
                nc.tensor.matmul(nabps[:, 0, :], w_pair[:cs, c, :], PT2[:cs, c, :],
                                 start=(c == 0), stop=(c == 1))
            for c, (c0, cs) in enumerate(CHUNKS):
                nc.tensor.matmul(nabps[:, 1, :], w_pair[:cs, c, :], HT2[:cs, c, :],
                                 start=(c == 0), stop=(c == 1))
            rnab = sc_pool.tile([L, 2, S], F32, tag="rnab")
            rsqrt(nc, sc_pool, rnab[:], nabps[:], "nab_s")
            rnab_b = sc_pool.tile([L, 2, S], BF16, tag="rnab_b")
            nc.scalar.copy(rnab_b[:], rnab[:])
            nc.tensor.transpose(psA[:S, 7, :L], rnab_b[:, 0, :], identb[:L, :L])
            tprf = mmA[:, 0, 136:156]
            nc.tensor.matmul(
                tprf[S : 2 * S], rnab_b[:, 1, :], identb[:L, :L],
                tile_position=(0, 64), start=True, stop=True,
            )
            rnT = persist.tile([128, 2, L], BF16, tag="rnT")
            nc.scalar.copy(rnT[:S, 0, :], psA[:S, 7, :L])
            nc.scalar.copy(rnT[S : 2 * S, 1, :], tprf[S : 2 * S])

            SH = big_pool.tile([128, 2, L, S], BF16, tag="SH")
            for c, (c0, cs) in enumerate(CHUNKS):
                ht_b = HT[:cs, c, :][:, None, :].broadcast_to([cs, L, S])
                w_b = w_pair[:cs, c, :][:, :, None].broadcast_to([cs, L, S])
                nc.gpsimd.tensor_tensor(
                    SH[:cs, c], ht_b, w_b, op=mybir.AluOpType.mult
                )
            st[(b, d)] = dict(
                P_natb=P_natb, H_natb=H_natb, PT=PT, HT=HT, PT2=PT2, HT2=HT2,
                MS_T=MS_T, rnT=rnT, rnab_b=rnab_b,
                VT1u=VT1u, GsPK=GsPK, SH=SH, psA=psA, mmA=mmA,
            )

        # pairwise (maxpool) pass structure: supplies for the first two
        # passes are emitted in phaseB (a full period before their scans),
        # so the scans are ready the moment phaseS starts; only pass 2's
        # supply rides inside phaseS, hidden under the big chunk scans.
        PMX_SIZES = (8, 7, 4, 1)
        PMX_OFF = (0, 8, 15, 19)

        def pmx_supply(v, u, lh):
            LH, l0 = PMX_SIZES[lh], PMX_OFF[lh]
            lsl = slice(l0, l0 + LH)
            SH, PT = v["SH"], v["PT"]
            comb = ps_nA.tile([128, 8, S], F32, tag="nA",
                              name=f"comb{u}_{lh}")[:, 0:LH, :]
            nA = comb[0:S]
            nA_flat = nA.rearrange("p l q -> p (l q)")
            for c, (c0, cs) in enumerate(CHUNKS):
                sh_flat = SH[:cs, c, lsl, :].rearrange("h l q -> h (l q)")
                nc.tensor.matmul(
                    nA_flat[:], PT[:cs, c, :], sh_flat[:],
                    start=(c == 0), stop=(c == 1),
                )
            nAs = big_pool.tile([S, 8, S], F32, tag="nAs",
                                name=f"nAs{u}_{lh}")[:, 0:LH, :]
            # two half-copies so the first transposes start sooner
            # (Act: GPSIMD cannot read PSUM)
            if LH > 2:
                nc.scalar.copy(nAs[:, 0:2, :], nA[:, 0:2, :])
                nc.scalar.copy(nAs[:, 2:LH, :], nA[:, 2:LH, :])
            else:
                nc.scalar.copy(nAs[:], nA[:])
            for l in range(LH):
                nc.tensor.matmul(
                    comb[S : 2 * S, l, :], nAs[:, l, :], identf[:],
                    tile_position=(0, 64),
                )
            v[f"comb{lh}"] = comb

        def pmx_scan(v, u, lh, pmxC):
            LH, l0 = PMX_SIZES[lh], PMX_OFF[lh]
            lsl = slice(l0, l0 + LH)
            comb = v.pop(f"comb{lh}")
            out_v = pmxC[:, lsl][:, :, None].broadcast_to([128, LH, S])
            mul_maxscan_seg(nc, out_v, comb[:], v["RAB"][:, lsl, :])

        def phaseB(b, d):
            """Staging DMAs + broadcasts, one pipeline stage after phaseA:
            every input here was computed a full period ago, so these DMAs
            never block the SP sequencer on data waits."""
            v = st[(b, d)]
            u = 2 * b + d
            att_dram = dram_pool.tile([2, S, S], BF16, tag="att_d")
            nc.sync.dma_start(
                att_dram[:].rearrange("d s q -> (d s) q"), v["GsPK"][:]
            )
            attP = ab_pool.tile([128, S, S], BF16, tag="attP")
            attH = ab_pool.tile([128, S, S], BF16, tag="attH")
            bcast_rows(attP[:], att_dram[0].rearrange("s q -> (s q)"), 128)
            bcast_rows(attH[:], att_dram[1].rearrange("s q -> (s q)"), 128)
            att1u = lg_pool.tile([128, S, S], BF16, tag="att1u", name=f"att1u{u}")
            bcast_sides(att1u[:], att_dram[:], S * S, S)
            # chunk2 (rows 192..200): stage the 8 v2T rows of each side and
            # replicate across the scan lanes (side-packed like GsPK)
            vt2_dram = dram_pool.tile([2, 8, S], BF16, tag="vt2_d")
            nc.sync.dma_start(vt2_dram[0], v["HT"][S : S + 8, 1, :])
            nc.sync.dma_start(vt2_dram[1], v["PT"][S : S + 8, 1, :])
            rep2 = lg_pool.tile([128, 8, S], BF16, tag="rep2", name=f"rep2{u}")
            bcast_sides(rep2[:], vt2_dram[:], 8 * S, S)
            rnab_dram = dram_pool.tile([L, 2, S], BF16, tag="rnab_d")
            nc.sync.dma_start(rnab_dram[:], v["rnab_b"][:])
            RAB = rb_pool.tile([128, L, S], BF16, tag="RAB")
            bcast_rows(RAB[0:S], rnab_dram[:, 1, :], S)
            bcast_rows(RAB[S : 2 * S], rnab_dram[:, 0, :], S)
            v.update(attP=attP, attH=attH, att1u=att1u, rep2=rep2, RAB=RAB)

        def phaseS(b, d):
            v = st[(b, d)]
            PT, HT = v["PT"], v["HT"]
            attP, attH = v["attP"], v["attH"]
            w_pair = w2t[2 + d]
            u = 2 * b + d

            pmxC = sc_pool.tile([128, L], F32, tag="pmxC", name=f"pmxC{u}")

            # ---- max-attentive chunk0 scans ----
            MhT = t_pool.tile([128, 2, S], BF16, tag="MhT", name=f"MhT{u}")
            MpT = t_pool.tile([128, 2, S], BF16, tag="MpT", name=f"MpT{u}")
            PG = S // 2
            for (VT, attB, MT) in ((HT, attP, MhT), (PT, attH, MpT)):
                for g in range(2):
                    sl = slice(g * PG, (g + 1) * PG)
                    vt_b = VT[:, 0, :][:, None, :].broadcast_to([128, PG, S])
                    out_v = MT[:, 0, sl][:, :, None].broadcast_to([128, PG, S])
                    mul_maxscan_seg(nc, out_v, vt_b, attB[:, sl, :])
            v["MhT"], v["MpT"] = MhT, MpT
            # chunk1b scan: rows 128..192, both sides
            MT1u = lg_pool.tile([128, S], BF16, tag="MT1u", name=f"MT1u{u}")
            in0 = v["VT1u"][:, None, :].broadcast_to([128, S, S])
            out_v = MT1u[:, :, None].broadcast_to([128, S, S])
            mul_maxscan_seg(nc, out_v, in0, v["att1u"][:])
            v["MT1u"] = MT1u
            # chunk2 page-dim scan: rows 192..200, both sides in one op
            # (lanes 0..64 = p with Gs rows, 64..128 = q with G2T rows)
            rep2 = v["rep2"]
            MT2s = lg_pool.tile([128, 8], BF16, tag="MT2s", name=f"MT2s{u}")
            in0 = v["GsPK"][:, None, :].broadcast_to([128, 8, S])
            out_v = MT2s[:, :, None].broadcast_to([128, 8, S])
            mul_maxscan_seg(nc, out_v, in0, rep2[:])
            v["MT2s"] = MT2s

            # pairwise passes: supply+scan interleaved in execution order —
            # pass p's supply runs on PE/Pool during the scans ahead of it
            # (ring-2 comb: pass p reuses pass p-2's buffer)
            for lh in range(len(PMX_SIZES)):
                pmx_supply(v, u, lh)
                pmx_scan(v, u, lh, pmxC)
            v["pmxC"] = pmxC

        def phaseT(b, d):
            stg_p, stg_h = stgs[b]
            v = st[(b, d)]
            PT, HT, PT2, HT2 = v["PT"], v["HT"], v["PT2"], v["HT2"]
            MS_hT = v["MS_T"][:, 0:2, :]
            MS_pT = v["MS_T"][:, 2:4, :]
            rnaT = v["rnT"][:, 0, :]
            MhT, MpT = v["MhT"], v["MpT"]
            pmxC = v["pmxC"]
            w_full = w2t[0 + d]
            w_mean = w2t[4 + d]
            w_amax = w2t[6 + d]
            so = 4 * d
            u = 2 * b + d

            # rows 128..192 from the per-unit packed group
            MT1u = v["MT1u"]
            nc.scalar.copy(MhT[0:S, 1, :], MT1u[0:S])
            # phase-T-local PSUM home: keeps phaseT out of the phaseA
            # psA/mmA rings so the deeper pipeline never WAR-stalls
            psT = ps_T.tile([128, S], F32, tag="psT", name=f"psT{u}")
            nc.tensor.matmul(psT[0:S, :], ident2[S : 2 * S, :],
                             MT1u[S : 2 * S], start=True, stop=True)
            nc.scalar.copy(MpT[0:S, 1, :], psT[0:S, :])
            # rows 192..200: one [128,8] -> [8,128] transpose covers both
            # sides (cols 0..64 = Mh rows, 64..128 = Mp rows)
            MT2s = v["MT2s"]
            c2t = psT[S : S + 8, :].bitcast(BF16)
            nc.tensor.transpose(c2t, MT2s[:, :], ident128[:])
            nc.scalar.copy(MhT[S : S + 8, 1, :], c2t[:, 0:S])
            nc.scalar.copy(MpT[S : S + 8, 1, :], c2t[:, S : 2 * S])

            mvhmax = sc_pool.tile([128, L], F32, tag="mvhmax", name=f"mvhm{u}")
            nc.gpsimd.tensor_tensor(
                mvhmax[S : 2 * S], pmxC[S : 2 * S],
                v["rnT"][S : 2 * S, 1, :], op=mybir.AluOpType.mult,
            )
            mvhmax_tiles[(b, d)] = mvhmax

            # ---- mp_match cosine accumulators: one PSUM bank ----
            blk = ps_blk.tile([S, 6, 4, L], F32, tag="blk")
            DOTp, NAp, NBp = blk[:, 0], blk[:, 1], blk[:, 2]
            DOTh, NAh, NBh = blk[:, 3], blk[:, 4], blk[:, 5]

            def mm_to(dst_ap, streams):
                for c, (c0, cs) in enumerate(CHUNKS):
                    nc.tensor.matmul(
                        dst_ap, streams[0](c), streams[1](c),
                        start=(c == 0), stop=(c == 1),
                    )

            def mp_stream(DOT, NA, NB, slot, v1T2, zT, z2T, w):
                mm_to(DOT[:, slot, :], (lambda c: zT[: CHUNKS[c][1], c, :],
                                        lambda c: w[: CHUNKS[c][1], c, :]))
                mm_to(NB[:, slot, :], (lambda c: z2T[: CHUNKS[c][1], c, :],
                                       lambda c: w[: CHUNKS[c][1], c, :]))

            def mp_na(NA, v1T2, ws):
                # NA = sum_h w^2 v1^2 for the three slots; per-slot
                # start/stop groups kept contiguous (interleaving
                # accumulation groups corrupts PSUM on HW)
                for slot, w in ws:
                    for c, (c0, cs) in enumerate(CHUNKS):
                        nc.tensor.matmul(
                            NA[:, slot, :], v1T2[:cs, c, :], w[:cs, c, :],
                            start=(c == 0), stop=(c == 1),
                        )

            def mk_z(v1T, v2T, tag, split=False):
                zT = t_pool.tile([128, 2, S], BF16, tag=f"zT_{tag}")
                z2T = t_pool.tile([128, 2, S], BF16, tag=f"z2T_{tag}")
                if split:
                    # per-chunk ops so chunk-0 streams start before the
                    # chunk-1 repack lands
                    for c in range(2):
                        nc.gpsimd.tensor_tensor(
                            zT[:, c, :], v1T[:, c, :], v2T[:, c, :],
                            op=mybir.AluOpType.mult,
                        )
                        nc.gpsimd.tensor_tensor(
                            z2T[:, c, :], v2T[:, c, :], v2T[:, c, :],
                            op=mybir.AluOpType.mult,
                        )
                else:
                    nc.gpsimd.tensor_tensor(
                        zT[:], v1T[:, :, :], v2T[:, :, :],
                        op=mybir.AluOpType.mult,
                    )
                    nc.gpsimd.tensor_tensor(
                        z2T[:], v2T[:, :, :], v2T[:, :, :],
                        op=mybir.AluOpType.mult,
                    )
                return zT, z2T

            class _ColBcast:
                def __init__(self, col2):
                    self.col2 = col2

                def __getitem__(self, idx):
                    sl, c, _ = idx
                    return self.col2[sl, c : c + 1].broadcast_to(
                        [self.col2[sl].shape[0], S]
                    )

            def mp_full(DOT, NA, NB, v1T, v1T2, vT, w, tag, j):
                zT = t_pool.tile([128, 2, S], BF16, tag=f"zT_f{tag}")
                col2 = sc_pool.tile([128, 2], BF16, tag=f"col2_{tag}")
                colf = sc_pool.tile([128, 2], F32, tag=f"colf_{tag}")
                nc.scalar.copy(colf[:], vT[:, :, j])
                for c, (c0, cs) in enumerate(CHUNKS):
                    nc.scalar.mul(
                        zT[:cs, c, :], v1T[:cs, c, :], colf[:cs, c : c + 1]
                    )
                nc.scalar.square(col2[:], vT[:, :, j])
                mp_stream(DOT, NA, NB, SLOT_FULL, v1T2, zT, _ColBcast(col2), w)

            j = 63 if d == 0 else 0
            ws3 = [(SLOT_AMAX, w_amax), (SLOT_FULL, w_full), (SLOT_MEAN, w_mean)]
            mp_na(NAp, PT2, ws3)
            mp_na(NAh, HT2, ws3)
            z, z2 = mk_z(PT, MhT, "pamax", split=True)
            mp_stream(DOTp, NAp, NBp, SLOT_AMAX, PT2, z, z2, w_amax)
            z, z2 = mk_z(HT, MpT, "hamax", split=True)
            mp_stream(DOTh, NAh, NBh, SLOT_AMAX, HT2, z, z2, w_amax)
            mp_full(DOTp, NAp, NBp, PT, PT2, HT, w_full, "p", j)
            mp_full(DOTh, NAh, NBh, HT, HT2, PT, w_full, "h", j)
            z, z2 = mk_z(PT, MS_hT, "pmean")
            mp_stream(DOTp, NAp, NBp, SLOT_MEAN, PT2, z, z2, w_mean)
            z, z2 = mk_z(HT, MS_pT, "hmean")
            mp_stream(DOTh, NAh, NBh, SLOT_MEAN, HT2, z, z2, w_mean)

            # ---- cosine tails: both sides batched per instruction ----
            nab2 = sc_pool.tile([S, 6, 4, L], F32, tag="nab2")
            nc.scalar.copy(nab2[:], blk[:])
            prod = sc_pool.tile([S, 8, L], F32, tag="prod")
            nc.gpsimd.tensor_tensor(
                prod[:].rearrange("s (a b) l -> s a b l", a=2),
                nab2[:, 1::3], nab2[:, 2::3], op=mybir.AluOpType.mult,
            )
            r = sc_pool.tile([S, 8, L], F32, tag="r")
            rsqrt(nc, sc_pool, r[:], prod[:], "dd")
            for side, stg in ((0, stg_p), (1, stg_h)):
                nc.gpsimd.tensor_tensor(
                    stg[:, so : so + 4, :], nab2[:, 3 * side],
                    r[:, 4 * side : 4 * side + 4, :],
                    op=mybir.AluOpType.mult,
                )
            nc.gpsimd.tensor_tensor(
                stg_p[:, so + SLOT_MAX, :], pmxC[0:S], rnaT[0:S],
                op=mybir.AluOpType.mult,
            )

        def emit_outputs(b):
            stg_p, stg_h = stgs[b]
            nc.sync.dma_start(mvp[b], stg_p[:].rearrange("s k l -> s (k l)"))
            # the over-p max rows are ready before the cosine tails; write the
            # rest of mvh around their columns so nothing serializes on them
            for d in range(2):
                so = 4 * d
                nc.sync.dma_start(
                    mvh[b, :, (so + SLOT_MAX) * L : (so + SLOT_MAX + 1) * L],
                    mvhmax_tiles[(b, d)][S : 2 * S],
                )
            for c0, c1 in ((0, SLOT_MAX), (SLOT_MAX + 1, 4 + SLOT_MAX), (4 + SLOT_MAX + 1, 8)):
                nc.sync.dma_start(
                    mvh[b, :, c0 * L : c1 * L],
                    stg_h[:, c0:c1, :].rearrange("s k l -> s (k l)"),
                )

        units = [(b, d) for b in range(BPC) for d in range(2)]
        NU = len(units)
        build_w2t(2)
        build_w2t(3)
        loadA(*units[0])
        loadA(*units[1])
        loadA(*units[2])
        phaseA(*units[0])
        for i in (0, 1, 4, 5, 6, 7):
            build_w2t(i)
        phaseA(*units[1])
        phaseB(*units[0])
        # steady state: load(k+3) | A(k+2) | B(k+1) | S(k) | T(k-1).
        # S/T are EMITTED first: the scheduler's priority heap follows
        # emission order per queue, so the ready scans of S(k) must sit
        # ahead of A(k+2)'s long-latency chains (recips etc.) on DVE.
        for k in range(NU):
            phaseS(*units[k])
            if k >= 1:
                phaseT(*units[k - 1])
            if k + 3 < NU:
                loadA(*units[k + 3])  # depth-3 lookahead
            if k + 2 < NU:
                phaseA(*units[k + 2])
            if k + 1 < NU:
                phaseB(*units[k + 1])
            # emit one stage after the pair's phaseT so the output DMAs
            # never hold the SP sequencer waiting on the cosine tails
            if k >= 2 and units[k - 2][1] == 1:
                emit_outputs(units[k - 2][0])
        phaseT(*units[-1])
        if units[-2][1] == 1:
            emit_outputs(units[-2][0])
        emit_outputs(units[-1][0])


_CACHE = {}


def _get_program():
    if "nc" not in _CACHE:
        _CACHE["nc"] = _build_core_program()
    return _CACHE["nc"]


def _run(inputs, trace=False):
    nc = _get_program()
    con_p = np.ascontiguousarray(np.asarray(inputs["con_p"], dtype=np.float32))
    con_h = np.ascontiguousarray(np.asarray(inputs["con_h"], dtype=np.float32))
    in_maps = []
    for c in range(NCORES):
        m = {
            "cp": con_p[c * BPC : (c + 1) * BPC],
            "ch": con_h[c * BPC : (c + 1) * BPC],
        }
        for i in range(8):
            m[f"w{i+1}"] = np.ascontiguousarray(
                np.asarray(inputs[f"mp_w{i+1}"], dtype=np.float32)
            )
        in_maps.append(m)
    res = bass_utils.run_bass_kernel_spmd(
        nc, in_maps, core_ids=list(range(NCORES)), trace=trace
    )
    mv_p = np.concatenate([r["mvp"] for r in res.results], axis=0)
    mv_h = np.concatenate([r["mvh"] for r in res.results], axis=0)
    return (mv_p, mv_h), res


def kernel(**inputs):
    out, _ = _run(inputs, trace=False)
    return out


def predicted_ns():
    """Cost-model (TimelineSim) predicted on-device duration in ns."""
    try:
        from trails.perfetto import LazyPerfetto

        def _noop(self, *a, **k):
            return 0

        for m in (
            "enable_explicit_ordering",
            "reserve_process_order",
            "add_counter",
        ):
            if not hasattr(LazyPerfetto, m):
                setattr(LazyPerfetto, m, _noop)
        from concourse.timeline_sim import TimelineSim

        return float(TimelineSim(_get_program(), trace=False).simulate())
    except Exception:
        return None


def run_profiled(**inputs):
    """Run on hardware; return (outputs, exec_time_ns).

    NTFF hardware tracing is unavailable under this axon client (no
    antenv.axon_hooks), so exec time falls back to the cost-model
    prediction, which the optimization loop was driven by.
    """
    try:
        out, res = _run(inputs, trace=True)
        ns = res.exec_time_ns
    except ModuleNotFoundError:
        out, _ = _run(inputs, trace=False)
        ns = None
    if ns is None:
        ns = predicted_ns()
    return out, ns



# revision 46
# speedup vs baseline: 1.0495x; 1.0165x over previous
"""Trainium2 Bass kernel for nn_MatchingLayer (BiMPM-style matching layer).

Full inputs: con_p/con_h (32, 64, 400) fp32 + 8 perspective weights (20, 200).
Returns (mv_p, mv_h), each (32, 64, 160) fp32.

Sharding: pure data parallel over batch — each of the 8 NeuronCores gets 4
examples; the weights are replicated.

Numerics: interior tensors are bf16 (PE runs 1 cyc/row vs 4 for fp32; DMA
broadcast volume halves); accumulations (PSUM), norms and the final cosine
tails stay fp32.  The overall output error budget is rel_l2 < 2e-2; measured
~1.7e-3 with this mix.

Structure: a 3-stage software pipeline over the 8 per-core (example,
direction) units — load(k+2) | phaseA(k+1) prep+staging+broadcasts |
phaseS(k) all max-scans + pairwise | phaseT(k-1) streams+tails+outputs.
The dominant cost is the fused mul+max DVE scan (cosine-weighted
max-attentive matching, ~1 cycle per element per lane).  Scan lanes are
packed: h-rows 0..128 at full 128 lanes per side, rows 128..192 of BOTH
sides of a unit in one 128-lane group (PE shift-matmuls at the legal
0/32/64 partition bases move data in/out), and rows 192..200 via
page-dimension scans fed straight from SBUF.  PSUM is budgeted to exactly
8 banks with phase-local ring tags so the pipeline stages decouple.

Key algebraic simplifications vs the reference (exactness notes):
 - cosine similarity cos(v1*w, v2*w) is invariant under any POSITIVE scalar
   rescaling of v2 rows.  The reference's attentive-mean branch divides
   att-sums by row sums through _dsmall (which divides by a POSITIVE number
   in all cases), and its attentive branches carry positive row scales
   1/||p_row||, 1/||h_row||.  All of those cancel in the final cosine, so we
   drop them and only keep the scale factors that sit INSIDE a sum or a max.
 - att[p,q] = G[p,q] * rn1[p] * rn2[q] with G = p_fw @ h_fw^T (the eps guard
   in the reference never triggers for this data -- denominators are O(100)
   vs eps=1e-8; validated in test.py).
"""

import numpy as np
import sys

for _p in ("/opt/trn_rl_repo",):
    if _p not in sys.path:
        sys.path.insert(0, _p)

import concourse.bass as bass
import concourse.bacc as bacc
import concourse.tile as tile
from concourse import mybir
from concourse.masks import make_identity
from concourse import bass_utils

# ---------------------------------------------------------------------------
# Custom DVE op: fused multiply + per-page-resetting max-scan.
# With a stride-0 innermost output AP this acts as a grouped max-reduce of
# in0*in1 in a single VectorE pass.
# ---------------------------------------------------------------------------
import concourse.dve_spec as dve_spec
from concourse.dve_spec import (
    Spec,
    Src0,
    Src1,
    AluOp,
    Scan,
    lower,
    _Stage,
    _scan_init,
)
from concourse.dve_ops import DveOp, OPS, CUSTOM_DVE_SPECS
from concourse.dve_uop import DveOpSpec

# identity registry of scans that reset at SUB_DIM_DONE
_SEG_SCAN_IDS: set[int] = set()

_orig_scan_overrides = dve_spec._scan_overrides


def _patched_scan_overrides(scans, node_stage):
    seed, step = _orig_scan_overrides(scans, node_stage)
    for scan in scans:
        if id(scan) in _SEG_SCAN_IDS:
            d = node_stage[scan]
            step[d] = _Stage(scan.op, _scan_init(scan), scan.expr)
    return seed, step


dve_spec._scan_overrides = _patched_scan_overrides


def _make_op():
    expr = Src0 * Src1
    sc = Scan(AluOp.MAX, expr)
    _SEG_SCAN_IDS.add(id(sc))
    body = sc

    def _ref(in0, in1, s0, s1, imm2):
        prod = (in0.astype(np.float32) * in1).astype(np.float32)
        return np.maximum.accumulate(prod, axis=-1)

    spec = Spec(body=body, reference=_ref)

    shas = {}
    for ver in ("v3", "v4"):
        try:
            uops = lower(spec, ver=ver)
            tmp = DveOpSpec(name="MUL_MAXSCAN_SEG", opcode=0, uops=uops, rd1_en=True)
            shas[ver] = tmp.sha(ver)
        except Exception:
            pass

    for _existing in OPS:
        if _existing.name == "MUL_MAXSCAN_SEG":
            return _existing
    op = DveOp(
        "MUL_MAXSCAN_SEG",
        spec,
        subdim=True,
        uops_sha=shas,
    )
    OPS.append(op)
    CUSTOM_DVE_SPECS[op.name] = op.spec
    import concourse.dve_ops as _dops
    _dops._SUB_OPCODE_FOR_NAME[op.name] = (
        _dops._CUSTOM_DVE_ROW_BASE + len(OPS) - 1
    )
    assert _dops._SUB_OPCODE_FOR_NAME[op.name] < 0x20
    return op


MUL_MAXSCAN_SEG = _make_op()


def mul_maxscan_seg(nc, out_ap, in0_ap, in1_ap):
    """out = per-page (innermost-dim-resetting) running max of in0*in1.
    All APs [P, S, N]."""
    return nc.vector._custom_dve(
        MUL_MAXSCAN_SEG, out=out_ap, in0=in0_ap, in1=in1_ap
    )


F32 = mybir.dt.float32
BF16 = mybir.dt.bfloat16
AF = mybir.ActivationFunctionType


def rsqrt(nc, sc_pool, out_ap, in_ap, tag):
    """1/sqrt(x): Act sqrt followed by the fast DVE reciprocal.
    (Act's Rsqrt/Abs_reciprocal_sqrt tables are inaccurate — measured
    2e-1 rel error — so the two-op form stays.)"""
    shp = list(in_ap.shape)
    tmp = sc_pool.tile(shp, F32, tag=tag)
    nc.scalar.sqrt(tmp[:], in_ap)
    return nc.vector.reciprocal_approx_fast(out_ap, tmp[:])

B, S, H, L = 32, 64, 200, 20
NCORES = 8
BPC = B // NCORES  # examples per core
CHUNKS = ((0, 128), (128, 72))  # h-dim split across partitions
SLOT_FULL, SLOT_MAX, SLOT_MEAN, SLOT_AMAX = 0, 1, 2, 3



def _build_core_program():
    nc = bacc.Bacc("TRN2", target_bir_lowering=False, debug=False)

    cp = nc.dram_tensor("cp", [BPC, S, 2 * H], F32, kind="ExternalInput").ap()
    ch = nc.dram_tensor("ch", [BPC, S, 2 * H], F32, kind="ExternalInput").ap()
    w_drams = [
        nc.dram_tensor(f"w{i+1}", [L, H], F32, kind="ExternalInput").ap()
        for i in range(8)
    ]
    mvp = nc.dram_tensor("mvp", [BPC, S, 8 * L], F32, kind="ExternalOutput").ap()
    mvh = nc.dram_tensor("mvh", [BPC, S, 8 * L], F32, kind="ExternalOutput").ap()

    with tile.TileContext(nc) as tc:
        _trace_kernel(nc, tc, cp, ch, w_drams, mvp, mvh)

    nc.compile()
    return nc


def _trace_kernel(nc, tc, cp, ch, w_drams, mvp, mvh):
    from contextlib import ExitStack

    ctx = ExitStack()
    with ctx:
        # ---------------- pools ----------------
        # PSUM budget (8 banks): psA bf16 x2, mmA f32 x2, mmB f32 x1,
        # blk x1, comb x2.
        const_pool = ctx.enter_context(tc.tile_pool(name="const", bufs=1))
        w2t_pool = ctx.enter_context(tc.tile_pool(name="w2t", bufs=1))
        in_pool = ctx.enter_context(tc.tile_pool(name="inp", bufs=4))
        t_pool = ctx.enter_context(tc.tile_pool(name="tlay", bufs=3))
        sc_pool = ctx.enter_context(tc.tile_pool(name="scal", bufs=4))
        big_pool = ctx.enter_context(tc.tile_pool(name="bigsb", bufs=3))
        gs_pool = ctx.enter_context(tc.tile_pool(name="gspk", bufs=3))
        ab_pool = ctx.enter_context(tc.tile_pool(name="abpool", bufs=3))
        stage_pool = ctx.enter_context(tc.tile_pool(name="stage", bufs=4))
        lg_pool = ctx.enter_context(tc.tile_pool(name="lgpool", bufs=3))
        ps_A = ctx.enter_context(tc.tile_pool(name="ps_A", bufs=2, space="PSUM"))
        ps_mmA = ctx.enter_context(tc.tile_pool(name="ps_mmA", bufs=1, space="PSUM"))
        ps_msT = ctx.enter_context(tc.tile_pool(name="ps_msT", bufs=1, space="PSUM"))
        ps_T = ctx.enter_context(tc.tile_pool(name="ps_T", bufs=1, space="PSUM"))
        ps_blk = ctx.enter_context(tc.tile_pool(name="ps_blk", bufs=1, space="PSUM"))

        ps_nA = ctx.enter_context(tc.tile_pool(name="ps_nA", bufs=2, space="PSUM"))
        dram_pool = ctx.enter_context(tc.tile_pool(name="drst", bufs=4, space="DRAM"))

        def bcast_rows(dst_ap, dram_ap, nparts, eng=None):
            """dst[(m, *free)] <- dram[*free] for all m (stride-0 partition dim)."""
            src = bass.AP(
                tensor=dram_ap.tensor,
                offset=dram_ap.offset,
                ap=[[0, nparts]] + [list(x) for x in dram_ap.ap],
            )
            (eng or nc.sync).dma_start(dst_ap, src)

        def bcast_sides(dst_ap, dram_ap2, n, nrep, eng=None):
            """dst[(g*nrep + m), 0:n] <- dram2[g, 0:n] for g in (0,1), all m:
            one DMA that broadcasts side g of a [2, n] DRAM block to
            partitions g*nrep..(g+1)*nrep."""
            src = bass.AP(
                tensor=dram_ap2.tensor,
                offset=dram_ap2.offset,
                ap=[[n, 2], [0, nrep], [1, n]],
            )
            (eng or nc.sync).dma_start(dst_ap, src)

        # ---------------- constants ----------------
        identb = const_pool.tile([64, 64], BF16, tag="identb")
        make_identity(nc, identb[:])
        identf = const_pool.tile([64, 64], F32, tag="identf")
        make_identity(nc, identf[:])
        ones_col = const_pool.tile([128, 1], BF16, tag="ones")
        nc.vector.memset(ones_col[:], 1.0)
        ident2 = const_pool.tile([128, 64], BF16, tag="ident2")
        nc.vector.memset(ident2[:], 0.0)
        make_identity(nc, ident2[64:128, :])
        ident128 = const_pool.tile([128, 128], BF16, tag="ident128")
        make_identity(nc, ident128[:])
        oneh = const_pool.tile([128, 4, 32], BF16, tag="oneh")
        nc.vector.memset(oneh[:], 0.0)
        for _s in range(4):
            make_identity(nc, oneh[64 : 64 + 8, _s, 8 * _s : 8 * _s + 8])

        # W2T[i]: squared weight, transposed: chunks (128,20),(72,20), bf16
        w2t = [None] * 8

        def build_w2t(i):
            w_nat = in_pool.tile([L, H], F32, tag="w_nat", name=f"w_nat{i}")
            nc.sync.dma_start(w_nat[:], w_drams[i])
            w2_nat = in_pool.tile([L, H], BF16, tag="w2_nat", name=f"w2n{i}")
            nc.gpsimd.tensor_tensor(
                w2_nat[:], w_nat[:], w_nat[:], op=mybir.AluOpType.mult
            )
            wt = w2t_pool.tile([128, 2, L], BF16, tag=f"w2t{i}", name=f"w2t{i}")
            tp = ps_A.tile([128, 16, 64], BF16, tag="psA", name=f"w2tp{i}")
            for c, (c0, cs) in enumerate(CHUNKS):
                nc.tensor.transpose(
                    tp[:cs, c, :L], w2_nat[:, c0 : c0 + cs], identb[:L, :L]
                )
            nc.scalar.copy(wt[:, 0, :], tp[:, 0, :L])
            nc.scalar.copy(wt[:72, 1, :], tp[:72, 1, :L])
            w2t[i] = wt

        # ---------------- interleaved phase A / phase B -----------------
        persist = ctx.enter_context(tc.tile_pool(name="persist", bufs=8))
        rb_pool = ctx.enter_context(tc.tile_pool(name="rbpool", bufs=3))

        st = {}
        mvhmax_tiles = {}
        stgs = {}
        for b in range(BPC):
            stgs[b] = (
                stage_pool.tile([S, 8, L], F32, tag="stg_p", name=f"stgp{b}"),
                stage_pool.tile([S, 8, L], F32, tag="stg_h", name=f"stgh{b}"),
            )

        loads = {}

        def loadA(b, d):
            off = d * H
            P_nat = in_pool.tile([S, H], F32, tag="P_nat", name=f"P_nat{b}{d}")
            H_nat = in_pool.tile([S, H], F32, tag="H_nat", name=f"H_nat{b}{d}")
            nc.sync.dma_start(P_nat[:], cp[b, :, off : off + H])
            nc.sync.dma_start(H_nat[:], ch[b, :, off : off + H])
            loads[(b, d)] = (P_nat, H_nat)

        def phaseA(b, d):
            w_pair = w2t[2 + d]
            u = 2 * b + d
            # per-unit PSUM homes (ring of 2 units)
            psA = ps_A.tile([128, 16, 64], BF16, tag="psA", name=f"psA{u}")
            mmA = ps_mmA.tile([128, 2, H], F32, tag="mmA", name=f"mmA{u}")

            P_nat, H_nat = loads.pop((b, d))
            P_natb = persist.tile([S, H], BF16, tag="P_natb")
            H_natb = persist.tile([S, H], BF16, tag="H_natb")
            nc.scalar.copy(P_natb[:], P_nat[:])
            nc.scalar.copy(H_natb[:], H_nat[:])

            # ---- transposed layouts + squares (bf16) ----
            # psA slots: 0,1 P chunks; 2,3 H chunks; 4 P rows 128..192 at
            # base 64 (for the packed chunk1b group)
            PHT = persist.tile([128, 4, S], BF16, tag="PHT")
            PHT2 = persist.tile([128, 4, S], BF16, tag="PHT2")
            PT, HT = PHT[:, 0:2, :], PHT[:, 2:4, :]
            PT2, HT2 = PHT2[:, 0:2, :], PHT2[:, 2:4, :]
            for c, (c0, cs) in enumerate(CHUNKS):
                nc.tensor.transpose(
                    psA[:cs, c, :S], P_natb[:, c0 : c0 + cs], identb[:]
                )
                nc.tensor.transpose(
                    psA[:cs, 2 + c, :S], H_natb[:, c0 : c0 + cs], identb[:]
                )
            nc.tensor.transpose(
                psA[S : 2 * S, 4, :S], P_natb[:, 128:192], identb[:]
            )
            nc.scalar.copy(PHT[:], psA[:, 0:4, :])
            nc.scalar.square(PHT2[:], psA[:, 0:4, :])
            VT1u = lg_pool.tile([128, S], BF16, tag="VT1u", name=f"VT1u{u}")
            nc.scalar.copy(VT1u[0:S], psA[0:S, 3, :])
            nc.scalar.copy(VT1u[S : 2 * S], psA[S : 2 * S, 4, :S])

            # ---- row norms -> rn (1/||row||), fp32 ----
            nsq = mmA[:S, :, 128:129].rearrange("s c one -> s (c one)")
            for c, (c0, cs) in enumerate(CHUNKS):
                nc.tensor.matmul(
                    nsq[:, 0:1], PT2[:cs, c, :], ones_col[:cs, :],
                    start=(c == 0), stop=(c == 1),
                )
            for c, (c0, cs) in enumerate(CHUNKS):
                nc.tensor.matmul(
                    nsq[:, 1:2], HT2[:cs, c, :], ones_col[:cs, :],
                    start=(c == 0), stop=(c == 1),
                )
            rn = sc_pool.tile([S, 2], F32, tag="rn")
            rsqrt(nc, sc_pool, rn[:], nsq[:], "nrm")
            rn1 = rn[:, 0:1]
            rn2 = rn[:, 1:2]

            # ---- G (p,q) and GT (q,p) ----
            gps = mmA[:S, :, 0:S]
            for c, (c0, cs) in enumerate(CHUNKS):
                nc.tensor.matmul(
                    gps[:, 0, :], PT[:cs, c, :], HT[:cs, c, :],
                    start=(c == 0), stop=(c == 1),
                )
            for c, (c0, cs) in enumerate(CHUNKS):
                nc.tensor.matmul(
                    gps[:, 1, :], HT[:cs, c, :], PT[:cs, c, :],
                    start=(c == 0), stop=(c == 1),
                )
            G2 = big_pool.tile([S, S], BF16, tag="G2")
            GsT = big_pool.tile([S, S], BF16, tag="GsT")
            nc.scalar.mul(G2[:], gps[:, 0, :], rn1)
            nc.scalar.mul(GsT[:], gps[:, 1, :], rn2)

            # transposes: Gs = GsT^T at parts 0..64, G2T = G2^T at parts
            # 64..128 -> ONE [128, S] side-packed tile (clean staging DMA +
            # direct chunk2 scan input)
            nc.tensor.transpose(psA[:S, 5, :S], GsT[:], identb[:])
            nc.tensor.transpose(
                psA[S : 2 * S, 5, :S], G2[:], identb[:], tile_position=(0, 64)
            )
            GsPK = gs_pool.tile([128, S], BF16, tag="GsPK")
            nc.scalar.copy(GsPK[:], psA[:, 5, :])

            # ---- attentive-mean sums, directly in transposed layout ----
            # MS_hT[h, s] = sum_q H[q, h] GsT[q, s]: stationary = the
            # natural-layout side (per chunk), moving = the att matrix.
            # Skips the msps matmuls + MS copy + 4 transposes entirely.
            MS_T = persist.tile([128, 4, S], BF16, tag="MS_T")
            msT = ps_msT.tile([128, 4, S], F32, tag="msT", name=f"msT{u}")
            for c, (c0, cs) in enumerate(CHUNKS):
                nc.tensor.matmul(
                    msT[:cs, 0 + c, :S], H_natb[:, c0 : c0 + cs], GsT[:],
                    start=True, stop=True,
                )
                nc.tensor.matmul(
                    msT[:cs, 2 + c, :S], P_natb[:, c0 : c0 + cs], G2[:],
                    start=True, stop=True,
                )
            nc.scalar.copy(MS_T[:], msT[:])

            # ---- pairwise norms rna/rnb ----
            nabps = mmA[:L, :, 136:200]
            for c, (c0, cs) in enumerate(CHUNKS):
                nc.tensor.matmul(nabps[:, 0, :], w_pair[:cs, c, :], PT2[:cs, c, :],
                                 start=(c == 0), stop=(c == 1))
            for c, (c0, cs) in enumerate(CHUNKS):
                nc.tensor.matmul(nabps[:, 1, :], w_pair[:cs, c, :], HT2[:cs, c, :],
                                 start=(c == 0), stop=(c == 1))
            rnab = sc_pool.tile([L, 2, S], F32, tag="rnab")
            rsqrt(nc, sc_pool, rnab[:], nabps[:], "nab_s")
            rnab_b = sc_pool.tile([L, 2, S], BF16, tag="rnab_b")
            nc.scalar.copy(rnab_b[:], rnab[:])
            nc.tensor.transpose(psA[:S, 7, :L], rnab_b[:, 0, :], identb[:L, :L])
            tprf = mmA[:, 0, 136:156]
            nc.tensor.matmul(
                tprf[S : 2 * S], rnab_b[:, 1, :], identb[:L, :L],
                tile_position=(0, 64), start=True, stop=True,
            )
            rnT = persist.tile([128, 2, L], BF16, tag="rnT")
            nc.scalar.copy(rnT[:S, 0, :], psA[:S, 7, :L])
            nc.scalar.copy(rnT[S : 2 * S, 1, :], tprf[S : 2 * S])

            SH = big_pool.tile([128, 2, L, S], BF16, tag="SH")
            for c, (c0, cs) in enumerate(CHUNKS):
                ht_b = HT[:cs, c, :][:, None, :].broadcast_to([cs, L, S])
                w_b = w_pair[:cs, c, :][:, :, None].broadcast_to([cs, L, S])
                nc.gpsimd.tensor_tensor(
                    SH[:cs, c], ht_b, w_b, op=mybir.AluOpType.mult
                )
            st[(b, d)] = dict(
                P_natb=P_natb, H_natb=H_natb, PT=PT, HT=HT, PT2=PT2, HT2=HT2,
                MS_T=MS_T, rnT=rnT, rnab_b=rnab_b,
                VT1u=VT1u, GsPK=GsPK, SH=SH, psA=psA, mmA=mmA,
            )

        # pairwise (maxpool) pass structure: supplies for the first two
        # passes are emitted in phaseB (a full period before their scans),
        # so the scans are ready the moment phaseS starts; only pass 2's
        # supply rides inside phaseS, hidden under the big chunk scans.
        PMX_SIZES = (8, 7, 4, 1)
        PMX_OFF = (0, 8, 15, 19)

        def pmx_supply(v, u, lh):
            LH, l0 = PMX_SIZES[lh], PMX_OFF[lh]
            lsl = slice(l0, l0 + LH)
            SH, PT = v["SH"], v["PT"]
            comb = ps_nA.tile([128, 8, S], F32, tag="nA",
                              name=f"comb{u}_{lh}")[:, 0:LH, :]
            nA = comb[0:S]
            nA_flat = nA.rearrange("p l q -> p (l q)")
            for c, (c0, cs) in enumerate(CHUNKS):
                sh_flat = SH[:cs, c, lsl, :].rearrange("h l q -> h (l q)")
                nc.tensor.matmul(
                    nA_flat[:], PT[:cs, c, :], sh_flat[:],
                    start=(c == 0), stop=(c == 1),
                )
            nAs = big_pool.tile([S, 8, S], F32, tag="nAs",
                                name=f"nAs{u}_{lh}")[:, 0:LH, :]
            # two half-copies so the first transposes start sooner
            # (Act: GPSIMD cannot read PSUM)
            if LH > 2:
                nc.scalar.copy(nAs[:, 0:2, :], nA[:, 0:2, :])
                nc.scalar.copy(nAs[:, 2:LH, :], nA[:, 2:LH, :])
            else:
                nc.scalar.copy(nAs[:], nA[:])
            for l in range(LH):
                nc.tensor.matmul(
                    comb[S : 2 * S, l, :], nAs[:, l, :], identf[:],
                    tile_position=(0, 64),
                )
            v[f"comb{lh}"] = comb

        def pmx_scan(v, u, lh, pmxC):
            LH, l0 = PMX_SIZES[lh], PMX_OFF[lh]
            lsl = slice(l0, l0 + LH)
            comb = v.pop(f"comb{lh}")
            out_v = pmxC[:, lsl][:, :, None].broadcast_to([128, LH, S])
            mul_maxscan_seg(nc, out_v, comb[:], v["RAB"][:, lsl, :])

        def phaseB(b, d):
            """Staging DMAs + broadcasts, one pipeline stage after phaseA:
            every input here was computed a full period ago, so these DMAs
            never block the SP sequencer on data waits."""
            v = st[(b, d)]
            u = 2 * b + d
            att_dram = dram_pool.tile([2, S, S], BF16, tag="att_d")
            nc.sync.dma_start(
                att_dram[:].rearrange("d s q -> (d s) q"), v["GsPK"][:]
            )
            attP = ab_pool.tile([128, S, S], BF16, tag="attP")
            attH = ab_pool.tile([128, S, S], BF16, tag="attH")
            bcast_rows(attP[:], att_dram[0].rearrange("s q -> (s q)"), 128)
            bcast_rows(attH[:], att_dram[1].rearrange("s q -> (s q)"), 128)
            att1u = lg_pool.tile([128, S, S], BF16, tag="att1u", name=f"att1u{u}")
            bcast_sides(att1u[:], att_dram[:], S * S, S)
            # chunk2 (rows 192..200): stage the 8 v2T rows of each side and
            # replicate across the scan lanes (side-packed like GsPK)
            vt2_dram = dram_pool.tile([2, 8, S], BF16, tag="vt2_d")
            nc.sync.dma_start(vt2_dram[0], v["HT"][S : S + 8, 1, :])
            nc.sync.dma_start(vt2_dram[1], v["PT"][S : S + 8, 1, :])
            rep2 = lg_pool.tile([128, 8, S], BF16, tag="rep2", name=f"rep2{u}")
            bcast_sides(rep2[:], vt2_dram[:], 8 * S, S)
            rnab_dram = dram_pool.tile([L, 2, S], BF16, tag="rnab_d")
            nc.sync.dma_start(rnab_dram[:], v["rnab_b"][:])
            RAB = rb_pool.tile([128, L, S], BF16, tag="RAB")
            bcast_rows(RAB[0:S], rnab_dram[:, 1, :], S)
            bcast_rows(RAB[S : 2 * S], rnab_dram[:, 0, :], S)
            v.update(attP=attP, attH=attH, att1u=att1u, rep2=rep2, RAB=RAB)

        def phaseS(b, d):
            v = st[(b, d)]
            PT, HT = v["PT"], v["HT"]
            attP, attH = v["attP"], v["attH"]
            w_pair = w2t[2 + d]
            u = 2 * b + d

            pmxC = sc_pool.tile([128, L], F32, tag="pmxC", name=f"pmxC{u}")

            # ---- max-attentive chunk0 scans ----
            MhT = t_pool.tile([128, 2, S], BF16, tag="MhT", name=f"MhT{u}")
            MpT = t_pool.tile([128, 2, S], BF16, tag="MpT", name=f"MpT{u}")
            PG = S // 2
            for (VT, attB, MT) in ((HT, attP, MhT), (PT, attH, MpT)):
                for g in range(2):
                    sl = slice(g * PG, (g + 1) * PG)
                    vt_b = VT[:, 0, :][:, None, :].broadcast_to([128, PG, S])
                    out_v = MT[:, 0, sl][:, :, None].broadcast_to([128, PG, S])
                    mul_maxscan_seg(nc, out_v, vt_b, attB[:, sl, :])
            v["MhT"], v["MpT"] = MhT, MpT
            # chunk1b scan: rows 128..192, both sides
            MT1u = lg_pool.tile([128, S], BF16, tag="MT1u", name=f"MT1u{u}")
            in0 = v["VT1u"][:, None, :].broadcast_to([128, S, S])
            out_v = MT1u[:, :, None].broadcast_to([128, S, S])
            mul_maxscan_seg(nc, out_v, in0, v["att1u"][:])
            v["MT1u"] = MT1u
            # chunk2 page-dim scan: rows 192..200, both sides in one op
            # (lanes 0..64 = p with Gs rows, 64..128 = q with G2T rows)
            rep2 = v["rep2"]
            MT2s = lg_pool.tile([128, 8], BF16, tag="MT2s", name=f"MT2s{u}")
            in0 = v["GsPK"][:, None, :].broadcast_to([128, 8, S])
            out_v = MT2s[:, :, None].broadcast_to([128, 8, S])
            mul_maxscan_seg(nc, out_v, in0, rep2[:])
            v["MT2s"] = MT2s

            # pairwise passes: supply+scan interleaved in execution order —
            # pass p's supply runs on PE/Pool during the scans ahead of it
            # (ring-2 comb: pass p reuses pass p-2's buffer)
            for lh in range(len(PMX_SIZES)):
                pmx_supply(v, u, lh)
                pmx_scan(v, u, lh, pmxC)
            v["pmxC"] = pmxC

        def phaseT(b, d):
            stg_p, stg_h = stgs[b]
            v = st[(b, d)]
            PT, HT, PT2, HT2 = v["PT"], v["HT"], v["PT2"], v["HT2"]
            MS_hT = v["MS_T"][:, 0:2, :]
            MS_pT = v["MS_T"][:, 2:4, :]
            rnaT = v["rnT"][:, 0, :]
            MhT, MpT = v["MhT"], v["MpT"]
            pmxC = v["pmxC"]
            w_full = w2t[0 + d]
            w_mean = w2t[4 + d]
            w_amax = w2t[6 + d]
            so = 4 * d
            u = 2 * b + d

            # rows 128..192 from the per-unit packed group
            MT1u = v["MT1u"]
            nc.scalar.copy(MhT[0:S, 1, :], MT1u[0:S])
            # phase-T-local PSUM home: keeps phaseT out of the phaseA
            # psA/mmA rings so the deeper pipeline never WAR-stalls
            psT = ps_T.tile([128, S], F32, tag="psT", name=f"psT{u}")
            nc.tensor.matmul(psT[0:S, :], ident2[S : 2 * S, :],
                             MT1u[S : 2 * S], start=True, stop=True)
            nc.scalar.copy(MpT[0:S, 1, :], psT[0:S, :])
            # rows 192..200: one [128,8] -> [8,128] transpose covers both
            # sides (cols 0..64 = Mh rows, 64..128 = Mp rows)
            MT2s = v["MT2s"]
            c2t = psT[S : S + 8, :].bitcast(BF16)
            nc.tensor.transpose(c2t, MT2s[:, :], ident128[:])
            nc.scalar.copy(MhT[S : S + 8, 1, :], c2t[:, 0:S])
            nc.scalar.copy(MpT[S : S + 8, 1, :], c2t[:, S : 2 * S])

            mvhmax = sc_pool.tile([128, L], F32, tag="mvhmax", name=f"mvhm{u}")
            nc.gpsimd.tensor_tensor(
                mvhmax[S : 2 * S], pmxC[S : 2 * S],
                v["rnT"][S : 2 * S, 1, :], op=mybir.AluOpType.mult,
            )
            mvhmax_tiles[(b, d)] = mvhmax

            # ---- mp_match cosine accumulators: one PSUM bank ----
            blk = ps_blk.tile([S, 6, 4, L], F32, tag="blk")
            DOTp, NAp, NBp = blk[:, 0], blk[:, 1], blk[:, 2]
            DOTh, NAh, NBh = blk[:, 3], blk[:, 4], blk[:, 5]

            def mm_to(dst_ap, streams):
                for c, (c0, cs) in enumerate(CHUNKS):
                    nc.tensor.matmul(
                        dst_ap, streams[0](c), streams[1](c),
                        start=(c == 0), stop=(c == 1),
                    )

            def mp_stream(DOT, NA, NB, slot, v1T2, zT, z2T, w):
                mm_to(DOT[:, slot, :], (lambda c: zT[: CHUNKS[c][1], c, :],
                                        lambda c: w[: CHUNKS[c][1], c, :]))
                mm_to(NB[:, slot, :], (lambda c: z2T[: CHUNKS[c][1], c, :],
                                       lambda c: w[: CHUNKS[c][1], c, :]))

            def mp_na(NA, v1T2, ws):
                # NA = sum_h w^2 v1^2 for the three slots; per-slot
                # start/stop groups kept contiguous (interleaving
                # accumulation groups corrupts PSUM on HW)
                for slot, w in ws:
                    for c, (c0, cs) in enumerate(CHUNKS):
                        nc.tensor.matmul(
                            NA[:, slot, :], v1T2[:cs, c, :], w[:cs, c, :],
                            start=(c == 0), stop=(c == 1),
                        )

            def mk_z(v1T, v2T, tag, split=False):
                zT = t_pool.tile([128, 2, S], BF16, tag=f"zT_{tag}")
                z2T = t_pool.tile([128, 2, S], BF16, tag=f"z2T_{tag}")
                if split:
                    # per-chunk ops so chunk-0 streams start before the
                    # chunk-1 repack lands
                    for c in range(2):
                        nc.gpsimd.tensor_tensor(
                            zT[:, c, :], v1T[:, c, :], v2T[:, c, :],
                            op=mybir.AluOpType.mult,
                        )
                        nc.gpsimd.tensor_tensor(
                            z2T[:, c, :], v2T[:, c, :], v2T[:, c, :],
                            op=mybir.AluOpType.mult,
                        )
                else:
                    nc.gpsimd.tensor_tensor(
                        zT[:], v1T[:, :, :], v2T[:, :, :],
                        op=mybir.AluOpType.mult,
                    )
                    nc.gpsimd.tensor_tensor(
                        z2T[:], v2T[:, :, :], v2T[:, :, :],
                        op=mybir.AluOpType.mult,
                    )
                return zT, z2T

            class _ColBcast:
                def __init__(self, col2):
                    self.col2 = col2

                def __getitem__(self, idx):
                    sl, c, _ = idx
                    return self.col2[sl, c : c + 1].broadcast_to(
                        [self.col2[sl].shape[0], S]
                    )

            def mp_full(DOT, NA, NB, v1T, v1T2, vT, w, tag, j):
                zT = t_pool.tile([128, 2, S], BF16, tag=f"zT_f{tag}")
                col2 = sc_pool.tile([128, 2], BF16, tag=f"col2_{tag}")
                colf = sc_pool.tile([128, 2], F32, tag=f"colf_{tag}")
                nc.scalar.copy(colf[:], vT[:, :, j])
                for c, (c0, cs) in enumerate(CHUNKS):
                    nc.scalar.mul(
                        zT[:cs, c, :], v1T[:cs, c, :], colf[:cs, c : c + 1]
                    )
                nc.scalar.square(col2[:], vT[:, :, j])
                mp_stream(DOT, NA, NB, SLOT_FULL, v1T2, zT, _ColBcast(col2), w)

            j = 63 if d == 0 else 0
            ws3 = [(SLOT_AMAX, w_amax), (SLOT_FULL, w_full), (SLOT_MEAN, w_mean)]
            mp_na(NAp, PT2, ws3)
            mp_na(NAh, HT2, ws3)
            z, z2 = mk_z(PT, MhT, "pamax", split=True)
            mp_stream(DOTp, NAp, NBp, SLOT_AMAX, PT2, z, z2, w_amax)
            z, z2 = mk_z(HT, MpT, "hamax", split=True)
            mp_stream(DOTh, NAh, NBh, SLOT_AMAX, HT2, z, z2, w_amax)
            mp_full(DOTp, NAp, NBp, PT, PT2, HT, w_full, "p", j)
            mp_full(DOTh, NAh, NBh, HT, HT2, PT, w_full, "h", j)
            z, z2 = mk_z(PT, MS_hT, "pmean")
            mp_stream(DOTp, NAp, NBp, SLOT_MEAN, PT2, z, z2, w_mean)
            z, z2 = mk_z(HT, MS_pT, "hmean")
            mp_stream(DOTh, NAh, NBh, SLOT_MEAN, HT2, z, z2, w_mean)

            # ---- cosine tails: both sides batched per instruction ----
            nab2 = sc_pool.tile([S, 6, 4, L], F32, tag="nab2")
            nc.scalar.copy(nab2[:], blk[:])
            prod = sc_pool.tile([S, 8, L], F32, tag="prod")
            nc.gpsimd.tensor_tensor(
                prod[:].rearrange("s (a b) l -> s a b l", a=2),
                nab2[:, 1::3], nab2[:, 2::3], op=mybir.AluOpType.mult,
            )
            r = sc_pool.tile([S, 8, L], F32, tag="r")
            rsqrt(nc, sc_pool, r[:], prod[:], "dd")
            for side, stg in ((0, stg_p), (1, stg_h)):
                nc.gpsimd.tensor_tensor(
                    stg[:, so : so + 4, :], nab2[:, 3 * side],
                    r[:, 4 * side : 4 * side + 4, :],
                    op=mybir.AluOpType.mult,
                )
            nc.gpsimd.tensor_tensor(
                stg_p[:, so + SLOT_MAX, :], pmxC[0:S], rnaT[0:S],
                op=mybir.AluOpType.mult,
            )

        def emit_outputs(b):
            stg_p, stg_h = stgs[b]
            nc.sync.dma_start(mvp[b], stg_p[:].rearrange("s k l -> s (k l)"))
            # the over-p max rows are ready before the cosine tails; write the
            # rest of mvh around their columns so nothing serializes on them
            for d in range(2):
                so = 4 * d
                nc.sync.dma_start(
                    mvh[b, :, (so + SLOT_MAX) * L : (so + SLOT_MAX + 1) * L],
                    mvhmax_tiles[(b, d)][S : 2 * S],
                )
            for c0, c1 in ((0, SLOT_MAX), (SLOT_MAX + 1, 4 + SLOT_MAX), (4 + SLOT_MAX + 1, 8)):
                nc.sync.dma_start(
                    mvh[b, :, c0 * L : c1 * L],
                    stg_h[:, c0:c1, :].rearrange("s k l -> s (k l)"),
                )

        units = [(b, d) for b in range(BPC) for d in range(2)]
        NU = len(units)
        build_w2t(2)
        build_w2t(3)
        loadA(*units[0])
        loadA(*units[1])
        loadA(*units[2])
        phaseA(*units[0])
        for i in (0, 1, 4, 5, 6, 7):
            build_w2t(i)
        phaseA(*units[1])
        phaseB(*units[0])
        # steady state: load(k+3) | A(k+2) | B(k+1) | S(k) | T(k-1).
        # S/T are EMITTED first: the scheduler's priority heap follows
        # emission order per queue, so the ready scans of S(k) must sit
        # ahead of A(k+2)'s long-latency chains (recips etc.) on DVE.
        for k in range(NU):
            phaseS(*units[k])
            if k >= 1:
                phaseT(*units[k - 1])
            if k + 3 < NU:
                loadA(*units[k + 3])  # depth-3 lookahead
            if k + 2 < NU:
                phaseA(*units[k + 2])
            if k + 1 < NU:
                phaseB(*units[k + 1])
            # emit one stage after the pair's phaseT so the output DMAs
            # never hold the SP sequencer waiting on the cosine tails
            if k >= 2 and units[k - 2][1] == 1:
                emit_outputs(units[k - 2][0])
        phaseT(*units[-1])
        if units[-2][1] == 1:
            emit_outputs(units[-2][0])
        emit_outputs(units[-1][0])


_CACHE = {}


def _get_program():
    if "nc" not in _CACHE:
        _CACHE["nc"] = _build_core_program()
    return _CACHE["nc"]


def _run(inputs, trace=False):
    nc = _get_program()
    con_p = np.ascontiguousarray(np.asarray(inputs["con_p"], dtype=np.float32))
    con_h = np.ascontiguousarray(np.asarray(inputs["con_h"], dtype=np.float32))
    in_maps = []
    for c in range(NCORES):
        m = {
            "cp": con_p[c * BPC : (c + 1) * BPC],
            "ch": con_h[c * BPC : (c + 1) * BPC],
        }
        for i in range(8):
            m[f"w{i+1}"] = np.ascontiguousarray(
                np.asarray(inputs[f"mp_w{i+1}"], dtype=np.float32)
            )
        in_maps.append(m)
    res = bass_utils.run_bass_kernel_spmd(
        nc, in_maps, core_ids=list(range(NCORES)), trace=trace
    )
    mv_p = np.concatenate([r["mvp"] for r in res.results], axis=0)
    mv_h = np.concatenate([r["mvh"] for r in res.results], axis=0)
    return (mv_p, mv_h), res


def kernel(**inputs):
    out, _ = _run(inputs, trace=False)
    return out


def predicted_ns():
    """Cost-model (TimelineSim) predicted on-device duration in ns."""
    try:
        from trails.perfetto import LazyPerfetto

        def _noop(self, *a, **k):
            return 0

        for m in (
            "enable_explicit_ordering",
            "reserve_process_order",
            "add_counter",
        ):
            if not hasattr(LazyPerfetto, m):
                setattr(LazyPerfetto, m, _noop)
        from concourse.timeline_sim import TimelineSim

        return float(TimelineSim(_get_program(), trace=False).simulate())
    except Exception:
        return None


def run_profiled(**inputs):
    """Run on hardware; return (outputs, exec_time_ns).

    NTFF hardware tracing is unavailable under this axon client (no
    antenv.axon_hooks), so exec time falls back to the cost-model
    prediction, which the optimization loop was driven by.
    """
    try:
        out, res = _run(inputs, trace=True)
        ns = res.exec_time_ns
    except ModuleNotFoundError:
        out, _ = _run(inputs, trace=False)
        ns = None
    if ns is None:
        ns = predicted_ns()
    return out, ns



# revision 56
# speedup vs baseline: 1.0848x; 1.0337x over previous
"""Trainium2 Bass kernel for nn_MatchingLayer (BiMPM-style matching layer).

Full inputs: con_p/con_h (32, 64, 400) fp32 + 8 perspective weights (20, 200).
Returns (mv_p, mv_h), each (32, 64, 160) fp32.

Sharding: pure data parallel over batch — each of the 8 NeuronCores gets 4
examples; the weights are replicated.

Numerics: interior tensors are bf16 (PE runs 1 cyc/row vs 4 for fp32; DMA
broadcast volume halves); accumulations (PSUM), norms and the final cosine
tails stay fp32.  The overall output error budget is rel_l2 < 2e-2; measured
~1.7e-3 with this mix.

Structure: a 3-stage software pipeline over the 8 per-core (example,
direction) units — load(k+2) | phaseA(k+1) prep+staging+broadcasts |
phaseS(k) all max-scans + pairwise | phaseT(k-1) streams+tails+outputs.
The dominant cost is the fused mul+max DVE scan (cosine-weighted
max-attentive matching, ~1 cycle per element per lane).  Scan lanes are
packed: h-rows 0..128 at full 128 lanes per side, rows 128..192 of BOTH
sides of a unit in one 128-lane group (PE shift-matmuls at the legal
0/32/64 partition bases move data in/out), and rows 192..200 via
page-dimension scans fed straight from SBUF.  PSUM is budgeted to exactly
8 banks with phase-local ring tags so the pipeline stages decouple.

Key algebraic simplifications vs the reference (exactness notes):
 - cosine similarity cos(v1*w, v2*w) is invariant under any POSITIVE scalar
   rescaling of v2 rows.  The reference's attentive-mean branch divides
   att-sums by row sums through _dsmall (which divides by a POSITIVE number
   in all cases), and its attentive branches carry positive row scales
   1/||p_row||, 1/||h_row||.  All of those cancel in the final cosine, so we
   drop them and only keep the scale factors that sit INSIDE a sum or a max.
 - att[p,q] = G[p,q] * rn1[p] * rn2[q] with G = p_fw @ h_fw^T (the eps guard
   in the reference never triggers for this data -- denominators are O(100)
   vs eps=1e-8; validated in test.py).
"""

import numpy as np
import sys

for _p in ("/opt/trn_rl_repo",):
    if _p not in sys.path:
        sys.path.insert(0, _p)

import concourse.bass as bass
import concourse.bacc as bacc
import concourse.tile as tile
from concourse import mybir
from concourse.masks import make_identity
from concourse import bass_utils

# ---------------------------------------------------------------------------
# Custom DVE op: fused multiply + per-page-resetting max-scan.
# With a stride-0 innermost output AP this acts as a grouped max-reduce of
# in0*in1 in a single VectorE pass.
# ---------------------------------------------------------------------------
import concourse.dve_spec as dve_spec
from concourse.dve_spec import (
    Spec,
    Src0,
    Src1,
    AluOp,
    Scan,
    lower,
    _Stage,
    _scan_init,
)
from concourse.dve_ops import DveOp, OPS, CUSTOM_DVE_SPECS
from concourse.dve_uop import DveOpSpec

# identity registry of scans that reset at SUB_DIM_DONE
_SEG_SCAN_IDS: set[int] = set()

_orig_scan_overrides = dve_spec._scan_overrides


def _patched_scan_overrides(scans, node_stage):
    seed, step = _orig_scan_overrides(scans, node_stage)
    for scan in scans:
        if id(scan) in _SEG_SCAN_IDS:
            d = node_stage[scan]
            step[d] = _Stage(scan.op, _scan_init(scan), scan.expr)
    return seed, step


dve_spec._scan_overrides = _patched_scan_overrides


def _make_op():
    expr = Src0 * Src1
    sc = Scan(AluOp.MAX, expr)
    _SEG_SCAN_IDS.add(id(sc))
    body = sc

    def _ref(in0, in1, s0, s1, imm2):
        prod = (in0.astype(np.float32) * in1).astype(np.float32)
        return np.maximum.accumulate(prod, axis=-1)

    spec = Spec(body=body, reference=_ref)

    shas = {}
    for ver in ("v3", "v4"):
        try:
            uops = lower(spec, ver=ver)
            tmp = DveOpSpec(name="MUL_MAXSCAN_SEG", opcode=0, uops=uops, rd1_en=True)
            shas[ver] = tmp.sha(ver)
        except Exception:
            pass

    for _existing in OPS:
        if _existing.name == "MUL_MAXSCAN_SEG":
            return _existing
    op = DveOp(
        "MUL_MAXSCAN_SEG",
        spec,
        subdim=True,
        uops_sha=shas,
    )
    OPS.append(op)
    CUSTOM_DVE_SPECS[op.name] = op.spec
    import concourse.dve_ops as _dops
    _dops._SUB_OPCODE_FOR_NAME[op.name] = (
        _dops._CUSTOM_DVE_ROW_BASE + len(OPS) - 1
    )
    assert _dops._SUB_OPCODE_FOR_NAME[op.name] < 0x20
    return op


MUL_MAXSCAN_SEG = _make_op()


def mul_maxscan_seg(nc, out_ap, in0_ap, in1_ap):
    """out = per-page (innermost-dim-resetting) running max of in0*in1.
    All APs [P, S, N]."""
    return nc.vector._custom_dve(
        MUL_MAXSCAN_SEG, out=out_ap, in0=in0_ap, in1=in1_ap
    )


F32 = mybir.dt.float32
BF16 = mybir.dt.bfloat16
AF = mybir.ActivationFunctionType


def rsqrt(nc, sc_pool, out_ap, in_ap, tag):
    """1/sqrt(x): Act sqrt followed by the fast DVE reciprocal.
    (Act's Rsqrt/Abs_reciprocal_sqrt tables are inaccurate — measured
    2e-1 rel error — so the two-op form stays.)"""
    shp = list(in_ap.shape)
    tmp = sc_pool.tile(shp, F32, tag=tag)
    nc.scalar.sqrt(tmp[:], in_ap)
    return nc.vector.reciprocal_approx_fast(out_ap, tmp[:])

B, S, H, L = 32, 64, 200, 20
NCORES = 8
BPC = B // NCORES  # examples per core
CHUNKS = ((0, 128), (128, 72))  # h-dim split across partitions
SLOT_FULL, SLOT_MAX, SLOT_MEAN, SLOT_AMAX = 0, 1, 2, 3



def _build_core_program():
    nc = bacc.Bacc("TRN2", target_bir_lowering=False, debug=False)

    cp = nc.dram_tensor("cp", [BPC, S, 2 * H], F32, kind="ExternalInput").ap()
    ch = nc.dram_tensor("ch", [BPC, S, 2 * H], F32, kind="ExternalInput").ap()
    w_drams = [
        nc.dram_tensor(f"w{i+1}", [L, H], F32, kind="ExternalInput").ap()
        for i in range(8)
    ]
    mvp = nc.dram_tensor("mvp", [BPC, S, 8 * L], F32, kind="ExternalOutput").ap()
    mvh = nc.dram_tensor("mvh", [BPC, S, 8 * L], F32, kind="ExternalOutput").ap()

    with tile.TileContext(nc) as tc:
        _trace_kernel(nc, tc, cp, ch, w_drams, mvp, mvh)

    nc.compile()
    return nc


def _trace_kernel(nc, tc, cp, ch, w_drams, mvp, mvh):
    from contextlib import ExitStack

    ctx = ExitStack()
    with ctx:
        # ---------------- pools ----------------
        # PSUM budget (8 banks): psA bf16 x2, mmA f32 x2, mmB f32 x1,
        # blk x1, comb x2.
        const_pool = ctx.enter_context(tc.tile_pool(name="const", bufs=1))
        w2t_pool = ctx.enter_context(tc.tile_pool(name="w2t", bufs=1))
        in_pool = ctx.enter_context(tc.tile_pool(name="inp", bufs=4))
        t_pool = ctx.enter_context(tc.tile_pool(name="tlay", bufs=3))
        sc_pool = ctx.enter_context(tc.tile_pool(name="scal", bufs=4))
        big_pool = ctx.enter_context(tc.tile_pool(name="bigsb", bufs=3))
        gs_pool = ctx.enter_context(tc.tile_pool(name="gspk", bufs=3))
        ab_pool = ctx.enter_context(tc.tile_pool(name="abpool", bufs=3))
        stage_pool = ctx.enter_context(tc.tile_pool(name="stage", bufs=4))
        lg_pool = ctx.enter_context(tc.tile_pool(name="lgpool", bufs=3))
        ps_A = ctx.enter_context(tc.tile_pool(name="ps_A", bufs=2, space="PSUM"))
        ps_mmA = ctx.enter_context(tc.tile_pool(name="ps_mmA", bufs=1, space="PSUM"))
        ps_msT = ctx.enter_context(tc.tile_pool(name="ps_msT", bufs=1, space="PSUM"))
        ps_T = ctx.enter_context(tc.tile_pool(name="ps_T", bufs=1, space="PSUM"))
        ps_blk = ctx.enter_context(tc.tile_pool(name="ps_blk", bufs=1, space="PSUM"))

        ps_nA = ctx.enter_context(tc.tile_pool(name="ps_nA", bufs=2, space="PSUM"))
        dram_pool = ctx.enter_context(tc.tile_pool(name="drst", bufs=4, space="DRAM"))

        def bcast_rows(dst_ap, dram_ap, nparts, eng=None):
            """dst[(m, *free)] <- dram[*free] for all m (stride-0 partition dim)."""
            src = bass.AP(
                tensor=dram_ap.tensor,
                offset=dram_ap.offset,
                ap=[[0, nparts]] + [list(x) for x in dram_ap.ap],
            )
            (eng or nc.sync).dma_start(dst_ap, src)

        def bcast_sides(dst_ap, dram_ap2, n, nrep, eng=None):
            """dst[(g*nrep + m), 0:n] <- dram2[g, 0:n] for g in (0,1), all m:
            one DMA that broadcasts side g of a [2, n] DRAM block to
            partitions g*nrep..(g+1)*nrep."""
            src = bass.AP(
                tensor=dram_ap2.tensor,
                offset=dram_ap2.offset,
                ap=[[n, 2], [0, nrep], [1, n]],
            )
            (eng or nc.sync).dma_start(dst_ap, src)

        # ---------------- constants ----------------
        identb = const_pool.tile([64, 64], BF16, tag="identb")
        make_identity(nc, identb[:])
        identf = const_pool.tile([64, 64], F32, tag="identf")
        make_identity(nc, identf[:])
        ones_col = const_pool.tile([128, 1], BF16, tag="ones")
        nc.vector.memset(ones_col[:], 1.0)
        ident2 = const_pool.tile([128, 64], BF16, tag="ident2")
        nc.vector.memset(ident2[:], 0.0)
        make_identity(nc, ident2[64:128, :])
        ident128 = const_pool.tile([128, 128], BF16, tag="ident128")
        make_identity(nc, ident128[:])
        oneh = const_pool.tile([128, 4, 32], BF16, tag="oneh")
        nc.vector.memset(oneh[:], 0.0)
        for _s in range(4):
            make_identity(nc, oneh[64 : 64 + 8, _s, 8 * _s : 8 * _s + 8])

        # W2T[i]: squared weight, transposed: chunks (128,20),(72,20), bf16
        w2t = [None] * 8

        def build_w2t(i):
            w_nat = in_pool.tile([L, H], F32, tag="w_nat", name=f"w_nat{i}")
            nc.sync.dma_start(w_nat[:], w_drams[i])
            w2_nat = in_pool.tile([L, H], BF16, tag="w2_nat", name=f"w2n{i}")
            nc.gpsimd.tensor_tensor(
                w2_nat[:], w_nat[:], w_nat[:], op=mybir.AluOpType.mult
            )
            wt = w2t_pool.tile([128, 2, L], BF16, tag=f"w2t{i}", name=f"w2t{i}")
            tp = ps_A.tile([128, 16, 64], BF16, tag="psA", name=f"w2tp{i}")
            for c, (c0, cs) in enumerate(CHUNKS):
                nc.tensor.transpose(
                    tp[:cs, c, :L], w2_nat[:, c0 : c0 + cs], identb[:L, :L]
                )
            nc.scalar.copy(wt[:, 0, :], tp[:, 0, :L])
            nc.scalar.copy(wt[:72, 1, :], tp[:72, 1, :L])
            w2t[i] = wt

        # ---------------- interleaved phase A / phase B -----------------
        persist = ctx.enter_context(tc.tile_pool(name="persist", bufs=8))
        rb_pool = ctx.enter_context(tc.tile_pool(name="rbpool", bufs=3))

        st = {}
        mvhmax_tiles = {}
        stgs = {}
        for b in range(BPC):
            stgs[b] = (
                stage_pool.tile([S, 8, L], F32, tag="stg_p", name=f"stgp{b}"),
                stage_pool.tile([S, 8, L], F32, tag="stg_h", name=f"stgh{b}"),
            )

        loads = {}

        def loadA(b, d):
            off = d * H
            P_nat = in_pool.tile([S, H], F32, tag="P_nat", name=f"P_nat{b}{d}")
            H_nat = in_pool.tile([S, H], F32, tag="H_nat", name=f"H_nat{b}{d}")
            nc.sync.dma_start(P_nat[:], cp[b, :, off : off + H])
            nc.sync.dma_start(H_nat[:], ch[b, :, off : off + H])
            loads[(b, d)] = (P_nat, H_nat)

        def phaseA(b, d):
            w_pair = w2t[2 + d]
            u = 2 * b + d
            # per-unit PSUM homes (ring of 2 units)
            psA = ps_A.tile([128, 16, 64], BF16, tag="psA", name=f"psA{u}")
            mmA = ps_mmA.tile([128, 2, H], F32, tag="mmA", name=f"mmA{u}")

            P_nat, H_nat = loads.pop((b, d))
            P_natb = persist.tile([S, H], BF16, tag="P_natb")
            H_natb = persist.tile([S, H], BF16, tag="H_natb")
            nc.scalar.copy(P_natb[:], P_nat[:])
            nc.scalar.copy(H_natb[:], H_nat[:])



            # ---- transposed layouts + squares (bf16) ----
            # psA slots: 0,1 P chunks; 2,3 H chunks; 4 P rows 128..192 at
            # base 64 (for the packed chunk1b group)
            PHT = persist.tile([128, 4, S], BF16, tag="PHT")
            PHT2 = persist.tile([128, 4, S], BF16, tag="PHT2")
            PT, HT = PHT[:, 0:2, :], PHT[:, 2:4, :]
            PT2, HT2 = PHT2[:, 0:2, :], PHT2[:, 2:4, :]
            for c, (c0, cs) in enumerate(CHUNKS):
                nc.tensor.transpose(
                    psA[:cs, c, :S], P_natb[:, c0 : c0 + cs], identb[:]
                )
                nc.tensor.transpose(
                    psA[:cs, 2 + c, :S], H_natb[:, c0 : c0 + cs], identb[:]
                )
            nc.tensor.transpose(
                psA[S : 2 * S, 4, :S], P_natb[:, 128:192], identb[:]
            )
            nc.scalar.copy(PHT[:], psA[:, 0:4, :])
            nc.scalar.square(PHT2[:], psA[:, 0:4, :])
            VT1u = lg_pool.tile([128, S], BF16, tag="VT1u", name=f"VT1u{u}")
            nc.scalar.copy(VT1u[0:S], psA[0:S, 3, :])
            nc.scalar.copy(VT1u[S : 2 * S], psA[S : 2 * S, 4, :S])

            # ---- row norms -> rn (1/||row||), fp32 ----
            nsq = mmA[:S, :, 128:129].rearrange("s c one -> s (c one)")
            for c, (c0, cs) in enumerate(CHUNKS):
                nc.tensor.matmul(
                    nsq[:, 0:1], PT2[:cs, c, :], ones_col[:cs, :],
                    start=(c == 0), stop=(c == 1),
                )
            for c, (c0, cs) in enumerate(CHUNKS):
                nc.tensor.matmul(
                    nsq[:, 1:2], HT2[:cs, c, :], ones_col[:cs, :],
                    start=(c == 0), stop=(c == 1),
                )
            rn = sc_pool.tile([S, 2], F32, tag="rn")
            rsqrt(nc, sc_pool, rn[:], nsq[:], "nrm")
            rn1 = rn[:, 0:1]
            rn2 = rn[:, 1:2]

            # ---- G (p,q) and GT (q,p) ----
            gps = mmA[:S, :, 0:S]
            for c, (c0, cs) in enumerate(CHUNKS):
                nc.tensor.matmul(
                    gps[:, 0, :], PT[:cs, c, :], HT[:cs, c, :],
                    start=(c == 0), stop=(c == 1),
                )
            for c, (c0, cs) in enumerate(CHUNKS):
                nc.tensor.matmul(
                    gps[:, 1, :], HT[:cs, c, :], PT[:cs, c, :],
                    start=(c == 0), stop=(c == 1),
                )
            G2 = big_pool.tile([S, S], BF16, tag="G2")
            GsT = big_pool.tile([S, S], BF16, tag="GsT")
            nc.scalar.mul(G2[:], gps[:, 0, :], rn1)
            nc.scalar.mul(GsT[:], gps[:, 1, :], rn2)

            # transposes: Gs = GsT^T at parts 0..64, G2T = G2^T at parts
            # 64..128 -> ONE [128, S] side-packed tile (clean staging DMA +
            # direct chunk2 scan input)
            nc.tensor.transpose(psA[:S, 5, :S], GsT[:], identb[:])
            nc.tensor.transpose(
                psA[S : 2 * S, 5, :S], G2[:], identb[:], tile_position=(0, 64)
            )
            GsPK = gs_pool.tile([128, S], BF16, tag="GsPK")
            nc.scalar.copy(GsPK[:], psA[:, 5, :])

            # ---- attentive-mean sums, directly in transposed layout ----
            # MS_hT[h, s] = sum_q H[q, h] GsT[q, s]: stationary = the
            # natural-layout side (per chunk), moving = the att matrix.
            # Skips the msps matmuls + MS copy + 4 transposes entirely.
            MS_T = persist.tile([128, 4, S], BF16, tag="MS_T")
            msT = ps_msT.tile([128, 4, S], F32, tag="msT", name=f"msT{u}")
            for c, (c0, cs) in enumerate(CHUNKS):
                nc.tensor.matmul(
                    msT[:cs, 0 + c, :S], H_natb[:, c0 : c0 + cs], GsT[:],
                    start=True, stop=True,
                )
                nc.tensor.matmul(
                    msT[:cs, 2 + c, :S], P_natb[:, c0 : c0 + cs], G2[:],
                    start=True, stop=True,
                )
            nc.scalar.copy(MS_T[:], msT[:])

            # ---- pairwise norms rna/rnb ----
            nabps = mmA[:L, :, 136:200]
            for c, (c0, cs) in enumerate(CHUNKS):
                nc.tensor.matmul(nabps[:, 0, :], w_pair[:cs, c, :], PT2[:cs, c, :],
                                 start=(c == 0), stop=(c == 1))
            for c, (c0, cs) in enumerate(CHUNKS):
                nc.tensor.matmul(nabps[:, 1, :], w_pair[:cs, c, :], HT2[:cs, c, :],
                                 start=(c == 0), stop=(c == 1))
            rnab = sc_pool.tile([L, 2, S], F32, tag="rnab")
            rsqrt(nc, sc_pool, rnab[:], nabps[:], "nab_s")
            rnab_b = sc_pool.tile([L, 2, S], BF16, tag="rnab_b")
            nc.scalar.copy(rnab_b[:], rnab[:])
            nc.tensor.transpose(psA[:S, 7, :L], rnab_b[:, 0, :], identb[:L, :L])
            tprf = mmA[:, 0, 136:156]
            nc.tensor.matmul(
                tprf[S : 2 * S], rnab_b[:, 1, :], identb[:L, :L],
                tile_position=(0, 64), start=True, stop=True,
            )
            rnT = persist.tile([128, 2, L], BF16, tag="rnT")
            nc.scalar.copy(rnT[:S, 0, :], psA[:S, 7, :L])
            nc.scalar.copy(rnT[S : 2 * S, 1, :], tprf[S : 2 * S])

            SH = big_pool.tile([128, 2, L, S], BF16, tag="SH")
            for c, (c0, cs) in enumerate(CHUNKS):
                ht_b = HT[:cs, c, :][:, None, :].broadcast_to([cs, L, S])
                w_b = w_pair[:cs, c, :][:, :, None].broadcast_to([cs, L, S])
                nc.gpsimd.tensor_tensor(
                    SH[:cs, c], ht_b, w_b, op=mybir.AluOpType.mult
                )
            st[(b, d)] = dict(
                P_natb=P_natb, H_natb=H_natb, PT=PT, HT=HT, PT2=PT2, HT2=HT2,
                MS_T=MS_T, rnT=rnT, rnab_b=rnab_b,
                VT1u=VT1u, GsPK=GsPK, SH=SH, psA=psA, mmA=mmA,
            )

        # pairwise (maxpool) pass structure: supplies for the first two
        # passes are emitted in phaseB (a full period before their scans),
        # so the scans are ready the moment phaseS starts; only pass 2's
        # supply rides inside phaseS, hidden under the big chunk scans.
        PMX_SIZES = (8, 7, 5)
        PMX_OFF = (0, 8, 15)

        def pmx_supply(v, u, lh):
            LH, l0 = PMX_SIZES[lh], PMX_OFF[lh]
            lsl = slice(l0, l0 + LH)
            SH, PT = v["SH"], v["PT"]
            comb = ps_nA.tile([128, 8, S], F32, tag="nA",
                              name=f"comb{u}_{lh}")[:, 0:LH, :]
            nA = comb[0:S]
            nA_flat = nA.rearrange("p l q -> p (l q)")
            for c, (c0, cs) in enumerate(CHUNKS):
                sh_flat = SH[:cs, c, lsl, :].rearrange("h l q -> h (l q)")
                nc.tensor.matmul(
                    nA_flat[:], PT[:cs, c, :], sh_flat[:],
                    start=(c == 0), stop=(c == 1),
                )
            nAs = big_pool.tile([S, 8, S], F32, tag="nAs",
                                name=f"nAs{u}_{lh}")[:, 0:LH, :]
            # two half-copies so the first transposes start sooner
            # (Act: GPSIMD cannot read PSUM)
            if LH > 2:
                nc.scalar.copy(nAs[:, 0:2, :], nA[:, 0:2, :])
                nc.scalar.copy(nAs[:, 2:LH, :], nA[:, 2:LH, :])
            else:
                nc.scalar.copy(nAs[:], nA[:])
            for l in range(LH):
                nc.tensor.matmul(
                    comb[S : 2 * S, l, :], nAs[:, l, :], identf[:],
                    tile_position=(0, 64),
                )
            v[f"comb{lh}"] = comb

        def pmx_scan(v, u, lh, pmxC):
            LH, l0 = PMX_SIZES[lh], PMX_OFF[lh]
            lsl = slice(l0, l0 + LH)
            comb = v.pop(f"comb{lh}")
            out_v = pmxC[:, lsl][:, :, None].broadcast_to([128, LH, S])
            mul_maxscan_seg(nc, out_v, comb[:], v["RAB"][:, lsl, :])

        def phaseB(b, d):
            """Staging DMAs + broadcasts, one pipeline stage after phaseA:
            every input here was computed a full period ago, so these DMAs
            never block the SP sequencer on data waits."""
            v = st[(b, d)]
            u = 2 * b + d
            att_dram = dram_pool.tile([2, S, S], BF16, tag="att_d")
            nc.sync.dma_start(
                att_dram[:].rearrange("d s q -> (d s) q"), v["GsPK"][:]
            )
            attP = ab_pool.tile([128, S, S], BF16, tag="attP")
            attH = ab_pool.tile([128, S, S], BF16, tag="attH")
            bcast_rows(attP[:], att_dram[0].rearrange("s q -> (s q)"), 128)
            bcast_rows(attH[:], att_dram[1].rearrange("s q -> (s q)"), 128)
            att1u = lg_pool.tile([128, S, S], BF16, tag="att1u", name=f"att1u{u}")
            bcast_sides(att1u[:], att_dram[:], S * S, S)
            # chunk2 (rows 192..200): stage the 8 v2T rows of each side and
            # replicate across the scan lanes (side-packed like GsPK)
            vt2_dram = dram_pool.tile([2, 8, S], BF16, tag="vt2_d")
            nc.sync.dma_start(vt2_dram[0], v["HT"][S : S + 8, 1, :])
            nc.sync.dma_start(vt2_dram[1], v["PT"][S : S + 8, 1, :])
            rep2 = lg_pool.tile([128, 8, S], BF16, tag="rep2", name=f"rep2{u}")
            bcast_sides(rep2[:], vt2_dram[:], 8 * S, S)
            rnab_dram = dram_pool.tile([L, 2, S], BF16, tag="rnab_d")
            nc.sync.dma_start(rnab_dram[:], v["rnab_b"][:])
            RAB = rb_pool.tile([128, L, S], BF16, tag="RAB")
            bcast_rows(RAB[0:S], rnab_dram[:, 1, :], S)
            bcast_rows(RAB[S : 2 * S], rnab_dram[:, 0, :], S)
            v.update(attP=attP, attH=attH, att1u=att1u, rep2=rep2, RAB=RAB)

        def phaseS(b, d):
            v = st[(b, d)]
            PT, HT = v["PT"], v["HT"]
            attP, attH = v["attP"], v["attH"]
            w_pair = w2t[2 + d]
            u = 2 * b + d

            pmxC = sc_pool.tile([128, L], F32, tag="pmxC", name=f"pmxC{u}")

            # ---- max-attentive chunk0 scans (one op per side) ----
            MhT = t_pool.tile([128, 2, S], BF16, tag="MhT", name=f"MhT{u}")
            MpT = t_pool.tile([128, 2, S], BF16, tag="MpT", name=f"MpT{u}")
            for (VT, attB, MT) in ((HT, attP, MhT), (PT, attH, MpT)):
                vt_b = VT[:, 0, :][:, None, :].broadcast_to([128, S, S])
                out_v = MT[:, 0, :][:, :, None].broadcast_to([128, S, S])
                mul_maxscan_seg(nc, out_v, vt_b, attB[:, :, :])
            v["MhT"], v["MpT"] = MhT, MpT
            # chunk1b scan: rows 128..192, both sides
            MT1u = lg_pool.tile([128, S], BF16, tag="MT1u", name=f"MT1u{u}")
            in0 = v["VT1u"][:, None, :].broadcast_to([128, S, S])
            out_v = MT1u[:, :, None].broadcast_to([128, S, S])
            mul_maxscan_seg(nc, out_v, in0, v["att1u"][:])
            v["MT1u"] = MT1u
            # chunk2 page-dim scan: rows 192..200, both sides in one op
            # (lanes 0..64 = p with Gs rows, 64..128 = q with G2T rows)
            rep2 = v["rep2"]
            MT2s = lg_pool.tile([128, 8], BF16, tag="MT2s", name=f"MT2s{u}")
            in0 = v["GsPK"][:, None, :].broadcast_to([128, 8, S])
            out_v = MT2s[:, :, None].broadcast_to([128, 8, S])
            mul_maxscan_seg(nc, out_v, in0, rep2[:])
            v["MT2s"] = MT2s

            # pairwise passes: supply+scan interleaved in execution order —
            # pass p's supply runs on PE/Pool during the scans ahead of it
            # (ring-2 comb: pass p reuses pass p-2's buffer)
            for lh in range(len(PMX_SIZES)):
                pmx_supply(v, u, lh)
                pmx_scan(v, u, lh, pmxC)
            v["pmxC"] = pmxC

        def phaseT(b, d):
            stg_p, stg_h = stgs[b]
            v = st[(b, d)]
            PT, HT, PT2, HT2 = v["PT"], v["HT"], v["PT2"], v["HT2"]
            MS_hT = v["MS_T"][:, 0:2, :]
            MS_pT = v["MS_T"][:, 2:4, :]
            rnaT = v["rnT"][:, 0, :]
            MhT, MpT = v["MhT"], v["MpT"]
            pmxC = v["pmxC"]
            w_full = w2t[0 + d]
            w_mean = w2t[4 + d]
            w_amax = w2t[6 + d]
            so = 4 * d
            u = 2 * b + d

            # rows 128..192 from the per-unit packed group
            MT1u = v["MT1u"]
            nc.scalar.copy(MhT[0:S, 1, :], MT1u[0:S])
            # phase-T-local PSUM home: keeps phaseT out of the phaseA
            # psA/mmA rings so the deeper pipeline never WAR-stalls
            psT = ps_T.tile([128, S], F32, tag="psT", name=f"psT{u}")
            nc.tensor.matmul(psT[0:S, :], ident2[S : 2 * S, :],
                             MT1u[S : 2 * S], start=True, stop=True)
            nc.scalar.copy(MpT[0:S, 1, :], psT[0:S, :])
            # rows 192..200: one [128,8] -> [8,128] transpose covers both
            # sides (cols 0..64 = Mh rows, 64..128 = Mp rows)
            MT2s = v["MT2s"]
            c2t = psT[S : S + 8, :].bitcast(BF16)
            nc.tensor.transpose(c2t, MT2s[:, :], ident128[:])
            nc.scalar.copy(MhT[S : S + 8, 1, :], c2t[:, 0:S])
            nc.scalar.copy(MpT[S : S + 8, 1, :], c2t[:, S : 2 * S])

            mvhmax = sc_pool.tile([128, L], F32, tag="mvhmax", name=f"mvhm{u}")
            nc.gpsimd.tensor_tensor(
                mvhmax[S : 2 * S], pmxC[S : 2 * S],
                v["rnT"][S : 2 * S, 1, :], op=mybir.AluOpType.mult,
            )
            mvhmax_tiles[(b, d)] = mvhmax

            # ---- mp_match cosine accumulators: one PSUM bank ----
            blk = ps_blk.tile([S, 6, 4, L], F32, tag="blk")
            DOTp, NAp, NBp = blk[:, 0], blk[:, 1], blk[:, 2]
            DOTh, NAh, NBh = blk[:, 3], blk[:, 4], blk[:, 5]

            def mm_to(dst_ap, streams):
                for c, (c0, cs) in enumerate(CHUNKS):
                    nc.tensor.matmul(
                        dst_ap, streams[0](c), streams[1](c),
                        start=(c == 0), stop=(c == 1),
                    )

            def mp_stream(DOT, NA, NB, slot, v1T2, zT, z2T, w):
                mm_to(DOT[:, slot, :], (lambda c: zT[: CHUNKS[c][1], c, :],
                                        lambda c: w[: CHUNKS[c][1], c, :]))
                mm_to(NB[:, slot, :], (lambda c: z2T[: CHUNKS[c][1], c, :],
                                       lambda c: w[: CHUNKS[c][1], c, :]))

            def mp_na(NA, v1T2, ws):
                # NA = sum_h w^2 v1^2 for the three slots; per-slot
                # start/stop groups kept contiguous (interleaving
                # accumulation groups corrupts PSUM on HW)
                for slot, w in ws:
                    for c, (c0, cs) in enumerate(CHUNKS):
                        nc.tensor.matmul(
                            NA[:, slot, :], v1T2[:cs, c, :], w[:cs, c, :],
                            start=(c == 0), stop=(c == 1),
                        )

            def mk_z(v1T, v2T, tag, split=False):
                zT = t_pool.tile([128, 2, S], BF16, tag=f"zT_{tag}")
                z2T = t_pool.tile([128, 2, S], BF16, tag=f"z2T_{tag}")
                if split:
                    # per-chunk ops so chunk-0 streams start before the
                    # chunk-1 repack lands
                    for c in range(2):
                        nc.gpsimd.tensor_tensor(
                            zT[:, c, :], v1T[:, c, :], v2T[:, c, :],
                            op=mybir.AluOpType.mult,
                        )
                        nc.gpsimd.tensor_tensor(
                            z2T[:, c, :], v2T[:, c, :], v2T[:, c, :],
                            op=mybir.AluOpType.mult,
                        )
                else:
                    nc.gpsimd.tensor_tensor(
                        zT[:], v1T[:, :, :], v2T[:, :, :],
                        op=mybir.AluOpType.mult,
                    )
                    nc.gpsimd.tensor_tensor(
                        z2T[:], v2T[:, :, :], v2T[:, :, :],
                        op=mybir.AluOpType.mult,
                    )
                return zT, z2T

            class _ColBcast:
                def __init__(self, col2):
                    self.col2 = col2

                def __getitem__(self, idx):
                    sl, c, _ = idx
                    return self.col2[sl, c : c + 1].broadcast_to(
                        [self.col2[sl].shape[0], S]
                    )

            def mp_full(DOT, NA, NB, v1T, v1T2, vT, w, tag, j):
                zT = t_pool.tile([128, 2, S], BF16, tag=f"zT_f{tag}")
                col2 = sc_pool.tile([128, 2], BF16, tag=f"col2_{tag}")
                colf = sc_pool.tile([128, 2], F32, tag=f"colf_{tag}")
                nc.scalar.copy(colf[:], vT[:, :, j])
                for c, (c0, cs) in enumerate(CHUNKS):
                    nc.scalar.mul(
                        zT[:cs, c, :], v1T[:cs, c, :], colf[:cs, c : c + 1]
                    )
                nc.scalar.square(col2[:], vT[:, :, j])
                mp_stream(DOT, NA, NB, SLOT_FULL, v1T2, zT, _ColBcast(col2), w)

            j = 63 if d == 0 else 0
            ws3 = [(SLOT_AMAX, w_amax), (SLOT_FULL, w_full), (SLOT_MEAN, w_mean)]
            mp_na(NAp, PT2, ws3)
            mp_na(NAh, HT2, ws3)
            z, z2 = mk_z(PT, MhT, "pamax", split=True)
            mp_stream(DOTp, NAp, NBp, SLOT_AMAX, PT2, z, z2, w_amax)
            z, z2 = mk_z(HT, MpT, "hamax", split=True)
            mp_stream(DOTh, NAh, NBh, SLOT_AMAX, HT2, z, z2, w_amax)
            mp_full(DOTp, NAp, NBp, PT, PT2, HT, w_full, "p", j)
            mp_full(DOTh, NAh, NBh, HT, HT2, PT, w_full, "h", j)
            z, z2 = mk_z(PT, MS_hT, "pmean")
            mp_stream(DOTp, NAp, NBp, SLOT_MEAN, PT2, z, z2, w_mean)
            z, z2 = mk_z(HT, MS_pT, "hmean")
            mp_stream(DOTh, NAh, NBh, SLOT_MEAN, HT2, z, z2, w_mean)

            # ---- cosine tails: both sides batched per instruction ----
            nab2 = sc_pool.tile([S, 6, 4, L], F32, tag="nab2")
            nc.scalar.copy(nab2[:], blk[:])
            prod = sc_pool.tile([S, 8, L], F32, tag="prod")
            nc.gpsimd.tensor_tensor(
                prod[:].rearrange("s (a b) l -> s a b l", a=2),
                nab2[:, 1::3], nab2[:, 2::3], op=mybir.AluOpType.mult,
            )
            r = sc_pool.tile([S, 8, L], F32, tag="r")
            rsqrt(nc, sc_pool, r[:], prod[:], "dd")
            for side, stg in ((0, stg_p), (1, stg_h)):
                nc.gpsimd.tensor_tensor(
                    stg[:, so : so + 4, :], nab2[:, 3 * side],
                    r[:, 4 * side : 4 * side + 4, :],
                    op=mybir.AluOpType.mult,
                )
            nc.gpsimd.tensor_tensor(
                stg_p[:, so + SLOT_MAX, :], pmxC[0:S], rnaT[0:S],
                op=mybir.AluOpType.mult,
            )

        def emit_outputs(b):
            stg_p, stg_h = stgs[b]
            nc.sync.dma_start(mvp[b], stg_p[:].rearrange("s k l -> s (k l)"))
            # the over-p max rows are ready before the cosine tails; write the
            # rest of mvh around their columns so nothing serializes on them
            for d in range(2):
                so = 4 * d
                nc.sync.dma_start(
                    mvh[b, :, (so + SLOT_MAX) * L : (so + SLOT_MAX + 1) * L],
                    mvhmax_tiles[(b, d)][S : 2 * S],
                )
            for c0, c1 in ((0, SLOT_MAX), (SLOT_MAX + 1, 4 + SLOT_MAX), (4 + SLOT_MAX + 1, 8)):
                nc.sync.dma_start(
                    mvh[b, :, c0 * L : c1 * L],
                    stg_h[:, c0:c1, :].rearrange("s k l -> s (k l)"),
                )

        units = [(b, d) for b in range(BPC) for d in range(2)]
        NU = len(units)
        build_w2t(2)
        build_w2t(3)
        loadA(*units[0])
        loadA(*units[1])
        loadA(*units[2])
        phaseA(*units[0])
        for i in (0, 1, 4, 5, 6, 7):
            build_w2t(i)
        phaseA(*units[1])
        phaseB(*units[0])
        # steady state: load(k+3) | A(k+2) | B(k+1) | S(k) | T(k-1).
        # S/T are EMITTED first: the scheduler's priority heap follows
        # emission order per queue, so the ready scans of S(k) must sit
        # ahead of A(k+2)'s long-latency chains (recips etc.) on DVE.
        for k in range(NU):
            phaseS(*units[k])
            if k >= 1:
                phaseT(*units[k - 1])
            if k + 3 < NU:
                loadA(*units[k + 3])  # depth-3 lookahead
            if k + 2 < NU:
                phaseA(*units[k + 2])
            if k + 1 < NU:
                phaseB(*units[k + 1])
            # emit one stage after the pair's phaseT so the output DMAs
            # never hold the SP sequencer waiting on the cosine tails
            if k >= 2 and units[k - 2][1] == 1:
                emit_outputs(units[k - 2][0])
        phaseT(*units[-1])
        if units[-2][1] == 1:
            emit_outputs(units[-2][0])
        emit_outputs(units[-1][0])


_CACHE = {}


def _get_program():
    if "nc" not in _CACHE:
        _CACHE["nc"] = _build_core_program()
    return _CACHE["nc"]


def _run(inputs, trace=False):
    nc = _get_program()
    con_p = np.ascontiguousarray(np.asarray(inputs["con_p"], dtype=np.float32))
    con_h = np.ascontiguousarray(np.asarray(inputs["con_h"], dtype=np.float32))
    in_maps = []
    for c in range(NCORES):
        m = {
            "cp": con_p[c * BPC : (c + 1) * BPC],
            "ch": con_h[c * BPC : (c + 1) * BPC],
        }
        for i in range(8):
            m[f"w{i+1}"] = np.ascontiguousarray(
                np.asarray(inputs[f"mp_w{i+1}"], dtype=np.float32)
            )
        in_maps.append(m)
    res = bass_utils.run_bass_kernel_spmd(
        nc, in_maps, core_ids=list(range(NCORES)), trace=trace
    )
    mv_p = np.concatenate([r["mvp"] for r in res.results], axis=0)
    mv_h = np.concatenate([r["mvh"] for r in res.results], axis=0)
    return (mv_p, mv_h), res


def kernel(**inputs):
    out, _ = _run(inputs, trace=False)
    return out


def predicted_ns():
    """Cost-model (TimelineSim) predicted on-device duration in ns."""
    try:
        from trails.perfetto import LazyPerfetto

        def _noop(self, *a, **k):
            return 0

        for m in (
            "enable_explicit_ordering",
            "reserve_process_order",
            "add_counter",
        ):
            if not hasattr(LazyPerfetto, m):
                setattr(LazyPerfetto, m, _noop)
        from concourse.timeline_sim import TimelineSim

        return float(TimelineSim(_get_program(), trace=False).simulate())
    except Exception:
        return None


def run_profiled(**inputs):
    """Run on hardware; return (outputs, exec_time_ns).

    NTFF hardware tracing is unavailable under this axon client (no
    antenv.axon_hooks), so exec time falls back to the cost-model
    prediction, which the optimization loop was driven by.
    """
    try:
        out, res = _run(inputs, trace=True)
        ns = res.exec_time_ns
    except ModuleNotFoundError:
        out, _ = _run(inputs, trace=False)
        ns = None
    if ns is None:
        ns = predicted_ns()
    return out, ns



# revision 89
# speedup vs baseline: 1.1106x; 1.0237x over previous
"""Trainium2 Bass kernel for nn_MatchingLayer (BiMPM-style matching layer).

Full inputs: con_p/con_h (32, 64, 400) fp32 + 8 perspective weights (20, 200).
Returns (mv_p, mv_h), each (32, 64, 160) fp32.

Sharding: pure data parallel over batch — each of the 8 NeuronCores gets 4
examples; the weights are replicated.

Numerics: interior tensors are bf16 (PE runs 1 cyc/row vs 4 for fp32; DMA
broadcast volume halves); accumulations (PSUM), norms and the final cosine
tails stay fp32.  The overall output error budget is rel_l2 < 2e-2; measured
~1.7e-3 with this mix.

Structure: a 3-stage software pipeline over the 8 per-core (example,
direction) units — load(k+2) | phaseA(k+1) prep+staging+broadcasts |
phaseS(k) all max-scans + pairwise | phaseT(k-1) streams+tails+outputs.
The dominant cost is the fused mul+max DVE scan (cosine-weighted
max-attentive matching, ~1 cycle per element per lane).  Scan lanes are
packed: h-rows 0..128 at full 128 lanes per side, rows 128..192 of BOTH
sides of a unit in one 128-lane group (PE shift-matmuls at the legal
0/32/64 partition bases move data in/out), and rows 192..200 via
page-dimension scans fed straight from SBUF.  PSUM is budgeted to exactly
8 banks with phase-local ring tags so the pipeline stages decouple.

Key algebraic simplifications vs the reference (exactness notes):
 - cosine similarity cos(v1*w, v2*w) is invariant under any POSITIVE scalar
   rescaling of v2 rows.  The reference's attentive-mean branch divides
   att-sums by row sums through _dsmall (which divides by a POSITIVE number
   in all cases), and its attentive branches carry positive row scales
   1/||p_row||, 1/||h_row||.  All of those cancel in the final cosine, so we
   drop them and only keep the scale factors that sit INSIDE a sum or a max.
 - att[p,q] = G[p,q] * rn1[p] * rn2[q] with G = p_fw @ h_fw^T (the eps guard
   in the reference never triggers for this data -- denominators are O(100)
   vs eps=1e-8; validated in test.py).
"""

import numpy as np
import sys

for _p in ("/opt/trn_rl_repo",):
    if _p not in sys.path:
        sys.path.insert(0, _p)

import concourse.bass as bass
import concourse.bacc as bacc
import concourse.tile as tile
from concourse import mybir
from concourse.masks import make_identity
from concourse import bass_utils

# ---------------------------------------------------------------------------
# Custom DVE op: fused multiply + per-page-resetting max-scan.
# With a stride-0 innermost output AP this acts as a grouped max-reduce of
# in0*in1 in a single VectorE pass.
# ---------------------------------------------------------------------------
import concourse.dve_spec as dve_spec
from concourse.dve_spec import (
    Spec,
    Src0,
    Src1,
    AluOp,
    Scan,
    lower,
    _Stage,
    _scan_init,
)
from concourse.dve_ops import DveOp, OPS, CUSTOM_DVE_SPECS
from concourse.dve_uop import DveOpSpec

# identity registry of scans that reset at SUB_DIM_DONE
_SEG_SCAN_IDS: set[int] = set()

_orig_scan_overrides = dve_spec._scan_overrides


def _patched_scan_overrides(scans, node_stage):
    seed, step = _orig_scan_overrides(scans, node_stage)
    for scan in scans:
        if id(scan) in _SEG_SCAN_IDS:
            d = node_stage[scan]
            step[d] = _Stage(scan.op, _scan_init(scan), scan.expr)
    return seed, step


dve_spec._scan_overrides = _patched_scan_overrides


def _make_op():
    expr = Src0 * Src1
    sc = Scan(AluOp.MAX, expr)
    _SEG_SCAN_IDS.add(id(sc))
    body = sc

    def _ref(in0, in1, s0, s1, imm2):
        prod = (in0.astype(np.float32) * in1).astype(np.float32)
        return np.maximum.accumulate(prod, axis=-1)

    spec = Spec(body=body, reference=_ref)

    shas = {}
    for ver in ("v3", "v4"):
        try:
            uops = lower(spec, ver=ver)
            tmp = DveOpSpec(name="MUL_MAXSCAN_SEG", opcode=0, uops=uops, rd1_en=True)
            shas[ver] = tmp.sha(ver)
        except Exception:
            pass

    for _existing in OPS:
        if _existing.name == "MUL_MAXSCAN_SEG":
            return _existing
    op = DveOp(
        "MUL_MAXSCAN_SEG",
        spec,
        subdim=True,
        uops_sha=shas,
    )
    OPS.append(op)
    CUSTOM_DVE_SPECS[op.name] = op.spec
    import concourse.dve_ops as _dops
    _dops._SUB_OPCODE_FOR_NAME[op.name] = (
        _dops._CUSTOM_DVE_ROW_BASE + len(OPS) - 1
    )
    assert _dops._SUB_OPCODE_FOR_NAME[op.name] < 0x20
    return op


MUL_MAXSCAN_SEG = _make_op()


def mul_maxscan_seg(nc, out_ap, in0_ap, in1_ap):
    """out = per-page (innermost-dim-resetting) running max of in0*in1.
    All APs [P, S, N]."""
    return nc.vector._custom_dve(
        MUL_MAXSCAN_SEG, out=out_ap, in0=in0_ap, in1=in1_ap
    )


F32 = mybir.dt.float32
BF16 = mybir.dt.bfloat16
AF = mybir.ActivationFunctionType


def rsqrt(nc, sc_pool, out_ap, in_ap, tag):
    """1/sqrt(x): Act sqrt followed by the fast DVE reciprocal.
    (Act's Rsqrt/Abs_reciprocal_sqrt tables are inaccurate — measured
    2e-1 rel error — so the two-op form stays.)"""
    shp = list(in_ap.shape)
    tmp = sc_pool.tile(shp, F32, tag=tag)
    nc.scalar.sqrt(tmp[:], in_ap)
    return nc.vector.reciprocal_approx_fast(out_ap, tmp[:])

B, S, H, L = 32, 64, 200, 20
NCORES = 8
BPC = B // NCORES  # examples per core
CHUNKS = ((0, 128), (128, 72))  # h-dim split across partitions
SLOT_FULL, SLOT_MAX, SLOT_MEAN, SLOT_AMAX = 0, 1, 2, 3



def _build_core_program():
    nc = bacc.Bacc("TRN2", target_bir_lowering=False, debug=False)

    cp = nc.dram_tensor("cp", [BPC, S, 2 * H], F32, kind="ExternalInput").ap()
    ch = nc.dram_tensor("ch", [BPC, S, 2 * H], F32, kind="ExternalInput").ap()
    w_drams = [
        nc.dram_tensor(f"w{i+1}", [L, H], F32, kind="ExternalInput").ap()
        for i in range(8)
    ]
    mvp = nc.dram_tensor("mvp", [BPC, S, 8 * L], F32, kind="ExternalOutput").ap()
    mvh = nc.dram_tensor("mvh", [BPC, S, 8 * L], F32, kind="ExternalOutput").ap()

    with tile.TileContext(nc) as tc:
        _trace_kernel(nc, tc, cp, ch, w_drams, mvp, mvh)

    nc.compile()
    return nc


def _trace_kernel(nc, tc, cp, ch, w_drams, mvp, mvh):
    from contextlib import ExitStack

    ctx = ExitStack()
    with ctx:
        # ---------------- pools ----------------
        # PSUM budget (8 banks): psA bf16 x2, mmA f32 x2, mmB f32 x1,
        # blk x1, comb x2.
        const_pool = ctx.enter_context(tc.tile_pool(name="const", bufs=1))
        w2t_pool = ctx.enter_context(tc.tile_pool(name="w2t", bufs=1))
        in_pool = ctx.enter_context(tc.tile_pool(name="inp", bufs=4))
        t_pool = ctx.enter_context(tc.tile_pool(name="tlay", bufs=3))
        sc_pool = ctx.enter_context(tc.tile_pool(name="scal", bufs=4))
        big_pool = ctx.enter_context(tc.tile_pool(name="bigsb", bufs=3))
        gs_pool = ctx.enter_context(tc.tile_pool(name="gspk", bufs=3))
        ab_pool = ctx.enter_context(tc.tile_pool(name="abpool", bufs=3))
        stage_pool = ctx.enter_context(tc.tile_pool(name="stage", bufs=4))
        lg_pool = ctx.enter_context(tc.tile_pool(name="lgpool", bufs=3))
        ps_A = ctx.enter_context(tc.tile_pool(name="ps_A", bufs=2, space="PSUM"))
        ps_mmA = ctx.enter_context(tc.tile_pool(name="ps_mmA", bufs=1, space="PSUM"))
        ps_msT = ctx.enter_context(tc.tile_pool(name="ps_msT", bufs=1, space="PSUM"))
        ps_T = ctx.enter_context(tc.tile_pool(name="ps_T", bufs=1, space="PSUM"))
        ps_blk = ctx.enter_context(tc.tile_pool(name="ps_blk", bufs=1, space="PSUM"))

        ps_nA = ctx.enter_context(tc.tile_pool(name="ps_nA", bufs=2, space="PSUM"))
        dram_pool = ctx.enter_context(tc.tile_pool(name="drst", bufs=4, space="DRAM"))

        def bcast_rows(dst_ap, dram_ap, nparts, eng=None):
            """dst[(m, *free)] <- dram[*free] for all m (stride-0 partition dim)."""
            src = bass.AP(
                tensor=dram_ap.tensor,
                offset=dram_ap.offset,
                ap=[[0, nparts]] + [list(x) for x in dram_ap.ap],
            )
            (eng or nc.sync).dma_start(dst_ap, src)

        def bcast_sides(dst_ap, dram_ap2, n, nrep, eng=None):
            """dst[(g*nrep + m), 0:n] <- dram2[g, 0:n] for g in (0,1), all m:
            one DMA that broadcasts side g of a [2, n] DRAM block to
            partitions g*nrep..(g+1)*nrep."""
            src = bass.AP(
                tensor=dram_ap2.tensor,
                offset=dram_ap2.offset,
                ap=[[n, 2], [0, nrep], [1, n]],
            )
            (eng or nc.sync).dma_start(dst_ap, src)

        # ---------------- constants ----------------
        identb = const_pool.tile([64, 64], BF16, tag="identb")
        make_identity(nc, identb[:])
        identf = const_pool.tile([64, 64], F32, tag="identf")
        make_identity(nc, identf[:])
        ones_col = const_pool.tile([128, 1], BF16, tag="ones")
        nc.vector.memset(ones_col[:], 1.0)
        ident2 = const_pool.tile([128, 64], BF16, tag="ident2")
        nc.vector.memset(ident2[:], 0.0)
        make_identity(nc, ident2[64:128, :])
        ident128 = const_pool.tile([128, 128], BF16, tag="ident128")
        make_identity(nc, ident128[:])
        oneh = const_pool.tile([128, 4, 32], BF16, tag="oneh")
        nc.vector.memset(oneh[:], 0.0)
        for _s in range(4):
            make_identity(nc, oneh[64 : 64 + 8, _s, 8 * _s : 8 * _s + 8])

        # W2T[i]: squared weight, transposed: chunks (128,20),(72,20), bf16
        w2t = [None] * 8

        def build_w2t(i):
            w_nat = in_pool.tile([L, H], F32, tag="w_nat", name=f"w_nat{i}")
            nc.sync.dma_start(w_nat[:], w_drams[i])
            w2_nat = in_pool.tile([L, H], BF16, tag="w2_nat", name=f"w2n{i}")
            nc.gpsimd.tensor_tensor(
                w2_nat[:], w_nat[:], w_nat[:], op=mybir.AluOpType.mult
            )
            wt = w2t_pool.tile([128, 2, L], BF16, tag=f"w2t{i}", name=f"w2t{i}")
            tp = ps_A.tile([128, 16, 64], BF16, tag="psA", name=f"w2tp{i}")
            for c, (c0, cs) in enumerate(CHUNKS):
                nc.tensor.transpose(
                    tp[:cs, c, :L], w2_nat[:, c0 : c0 + cs], identb[:L, :L]
                )
            # PSUM->SBUF repack on DVE: Act is the ramp-critical queue
            # (these 16 copies were starving unit 0's phaseA spine) and
            # DVE is idle until the first broadcast lands anyway
            nc.vector.tensor_scalar_add(wt[:, 0, :], tp[:, 0, :L], 0.0)
            nc.vector.tensor_scalar_add(wt[:72, 1, :], tp[:72, 1, :L], 0.0)
            w2t[i] = wt

        # ---------------- interleaved phase A / phase B -----------------
        persist = ctx.enter_context(tc.tile_pool(name="persist", bufs=8))
        rb_pool = ctx.enter_context(tc.tile_pool(name="rbpool", bufs=3))

        st = {}
        mvhmax_tiles = {}
        stgs = {}
        for b in range(BPC):
            stgs[b] = (
                stage_pool.tile([S, 8, L], F32, tag="stg_p", name=f"stgp{b}"),
                stage_pool.tile([S, 8, L], F32, tag="stg_h", name=f"stgh{b}"),
            )

        loads = {}

        def loadA(b, d):
            off = d * H
            P_nat = in_pool.tile([S, H], F32, tag="P_nat", name=f"P_nat{b}{d}")
            H_nat = in_pool.tile([S, H], F32, tag="H_nat", name=f"H_nat{b}{d}")
            nc.sync.dma_start(P_nat[:], cp[b, :, off : off + H])
            nc.sync.dma_start(H_nat[:], ch[b, :, off : off + H])
            loads[(b, d)] = (P_nat, H_nat)

        def phaseA(b, d, ramp=False):
            w_pair = w2t[2 + d]
            u = 2 * b + d

            # ramp variant: the Act queue is the prologue bottleneck, and
            # DVE idles until the first broadcast lands — push the spine's
            # elementwise ops there for the first units
            def cpy(dst, srcv):
                if ramp:
                    nc.vector.tensor_scalar_add(dst, srcv, 0.0)
                else:
                    nc.scalar.copy(dst, srcv)

            def sqr(dst, srcv):
                if ramp:
                    nc.vector.tensor_tensor(dst, srcv, srcv,
                                            op=mybir.AluOpType.mult)
                else:
                    nc.scalar.square(dst, srcv)

            def mulc(dst, srcv, col):
                if ramp:
                    nc.vector.tensor_scalar(dst, srcv, col, None,
                                            op0=mybir.AluOpType.mult)
                else:
                    nc.scalar.mul(dst, srcv, col)
            # per-unit PSUM homes (ring of 2 units)
            psA = ps_A.tile([128, 16, 64], BF16, tag="psA", name=f"psA{u}")
            mmA = ps_mmA.tile([128, 2, H], F32, tag="mmA", name=f"mmA{u}")

            P_nat, H_nat = loads.pop((b, d))
            P_natb = persist.tile([S, H], BF16, tag="P_natb")
            H_natb = persist.tile([S, H], BF16, tag="H_natb")
            cpy(P_natb[:], P_nat[:])
            cpy(H_natb[:], H_nat[:])



            # ---- transposed layouts + squares (bf16) ----
            # psA slots: 0,1 P chunks; 2,3 H chunks; 4 P rows 128..192 at
            # base 64 (for the packed chunk1b group)
            PHT = persist.tile([128, 4, S], BF16, tag="PHT")
            PHT2 = persist.tile([128, 4, S], BF16, tag="PHT2")
            PT, HT = PHT[:, 0:2, :], PHT[:, 2:4, :]
            PT2, HT2 = PHT2[:, 0:2, :], PHT2[:, 2:4, :]
            for c, (c0, cs) in enumerate(CHUNKS):
                nc.tensor.transpose(
                    psA[:cs, c, :S], P_natb[:, c0 : c0 + cs], identb[:]
                )
                nc.tensor.transpose(
                    psA[:cs, 2 + c, :S], H_natb[:, c0 : c0 + cs], identb[:]
                )
            nc.tensor.transpose(
                psA[S : 2 * S, 4, :S], P_natb[:, 128:192], identb[:]
            )
            cpy(PHT[:], psA[:, 0:4, :])
            sqr(PHT2[:], psA[:, 0:4, :])
            VT1u = lg_pool.tile([128, S], BF16, tag="VT1u", name=f"VT1u{u}")
            cpy(VT1u[0:S], psA[0:S, 3, :])
            cpy(VT1u[S : 2 * S], psA[S : 2 * S, 4, :S])

            # ---- row norms -> rn (1/||row||), fp32 ----
            nsq = mmA[:S, :, 128:129].rearrange("s c one -> s (c one)")
            for c, (c0, cs) in enumerate(CHUNKS):
                nc.tensor.matmul(
                    nsq[:, 0:1], PT2[:cs, c, :], ones_col[:cs, :],
                    start=(c == 0), stop=(c == 1),
                )
            for c, (c0, cs) in enumerate(CHUNKS):
                nc.tensor.matmul(
                    nsq[:, 1:2], HT2[:cs, c, :], ones_col[:cs, :],
                    start=(c == 0), stop=(c == 1),
                )
            rn = sc_pool.tile([S, 2], F32, tag="rn")
            rsqrt(nc, sc_pool, rn[:], nsq[:], "nrm")
            rn1 = rn[:, 0:1]
            rn2 = rn[:, 1:2]

            # ---- G (p,q) and GT (q,p) ----
            gps = mmA[:S, :, 0:S]
            for c, (c0, cs) in enumerate(CHUNKS):
                nc.tensor.matmul(
                    gps[:, 0, :], PT[:cs, c, :], HT[:cs, c, :],
                    start=(c == 0), stop=(c == 1),
                )
            for c, (c0, cs) in enumerate(CHUNKS):
                nc.tensor.matmul(
                    gps[:, 1, :], HT[:cs, c, :], PT[:cs, c, :],
                    start=(c == 0), stop=(c == 1),
                )
            G2 = big_pool.tile([S, S], BF16, tag="G2")
            GsT = big_pool.tile([S, S], BF16, tag="GsT")
            mulc(G2[:], gps[:, 0, :], rn1)
            mulc(GsT[:], gps[:, 1, :], rn2)

            # transposes: Gs = GsT^T at parts 0..64, G2T = G2^T at parts
            # 64..128 -> ONE [128, S] side-packed tile (clean staging DMA +
            # direct chunk2 scan input)
            nc.tensor.transpose(psA[:S, 5, :S], GsT[:], identb[:])
            nc.tensor.transpose(
                psA[S : 2 * S, 5, :S], G2[:], identb[:], tile_position=(0, 64)
            )
            GsPK = gs_pool.tile([128, S], BF16, tag="GsPK")
            cpy(GsPK[:], psA[:, 5, :])

            # ---- attentive-mean sums, directly in transposed layout ----
            # MS_hT[h, s] = sum_q H[q, h] GsT[q, s]: stationary = the
            # natural-layout side (per chunk), moving = the att matrix.
            # Skips the msps matmuls + MS copy + 4 transposes entirely.
            MS_T = persist.tile([128, 4, S], BF16, tag="MS_T")
            msT = ps_msT.tile([128, 4, S], F32, tag="msT", name=f"msT{u}")
            for c, (c0, cs) in enumerate(CHUNKS):
                nc.tensor.matmul(
                    msT[:cs, 0 + c, :S], H_natb[:, c0 : c0 + cs], GsT[:],
                    start=True, stop=True,
                )
                nc.tensor.matmul(
                    msT[:cs, 2 + c, :S], P_natb[:, c0 : c0 + cs], G2[:],
                    start=True, stop=True,
                )
            nc.scalar.copy(MS_T[:], msT[:])

            # ---- pairwise norms rna/rnb ----
            nabps = mmA[:L, :, 136:200]
            for c, (c0, cs) in enumerate(CHUNKS):
                nc.tensor.matmul(nabps[:, 0, :], w_pair[:cs, c, :], PT2[:cs, c, :],
                                 start=(c == 0), stop=(c == 1))
            for c, (c0, cs) in enumerate(CHUNKS):
                nc.tensor.matmul(nabps[:, 1, :], w_pair[:cs, c, :], HT2[:cs, c, :],
                                 start=(c == 0), stop=(c == 1))
            rnab = sc_pool.tile([L, 2, S], F32, tag="rnab")
            rsqrt(nc, sc_pool, rnab[:], nabps[:], "nab_s")
            rnab_b = sc_pool.tile([L, 2, S], BF16, tag="rnab_b")
            nc.scalar.copy(rnab_b[:], rnab[:])
            nc.tensor.transpose(psA[:S, 7, :L], rnab_b[:, 0, :], identb[:L, :L])
            tprf = mmA[:, 0, 136:156]
            nc.tensor.matmul(
                tprf[S : 2 * S], rnab_b[:, 1, :], identb[:L, :L],
                tile_position=(0, 64), start=True, stop=True,
            )
            rnT = persist.tile([128, 2, L], BF16, tag="rnT")
            nc.scalar.copy(rnT[:S, 0, :], psA[:S, 7, :L])
            nc.scalar.copy(rnT[S : 2 * S, 1, :], tprf[S : 2 * S])

            SH = big_pool.tile([128, 2, L, S], BF16, tag="SH")
            for c, (c0, cs) in enumerate(CHUNKS):
                ht_b = HT[:cs, c, :][:, None, :].broadcast_to([cs, L, S])
                w_b = w_pair[:cs, c, :][:, :, None].broadcast_to([cs, L, S])
                nc.gpsimd.tensor_tensor(
                    SH[:cs, c], ht_b, w_b, op=mybir.AluOpType.mult
                )
            st[(b, d)] = dict(
                P_natb=P_natb, H_natb=H_natb, PT=PT, HT=HT, PT2=PT2, HT2=HT2,
                MS_T=MS_T, rnT=rnT, rnab_b=rnab_b,
                VT1u=VT1u, GsPK=GsPK, SH=SH, psA=psA, mmA=mmA,
            )

        # pairwise (maxpool) pass structure: supplies for the first two
        # passes are emitted in phaseB (a full period before their scans),
        # so the scans are ready the moment phaseS starts; only pass 2's
        # supply rides inside phaseS, hidden under the big chunk scans.
        PMX_SIZES = (8, 7, 5)
        PMX_OFF = (0, 8, 15)

        def pmx_supply(v, u, lh):
            LH, l0 = PMX_SIZES[lh], PMX_OFF[lh]
            lsl = slice(l0, l0 + LH)
            SH, PT = v["SH"], v["PT"]
            comb = ps_nA.tile([128, 8, S], F32, tag="nA",
                              name=f"comb{u}_{lh}")[:, 0:LH, :]
            nA = comb[0:S]
            nA_flat = nA.rearrange("p l q -> p (l q)")
            for c, (c0, cs) in enumerate(CHUNKS):
                sh_flat = SH[:cs, c, lsl, :].rearrange("h l q -> h (l q)")
                nc.tensor.matmul(
                    nA_flat[:], PT[:cs, c, :], sh_flat[:],
                    start=(c == 0), stop=(c == 1),
                )
            nAs = big_pool.tile([S, 8, S], F32, tag="nAs",
                                name=f"nAs{u}_{lh}")[:, 0:LH, :]
            # two half-copies so the first transposes start sooner
            # (Act: GPSIMD cannot read PSUM); high priority so they beat
            # the phaseT tail ops through the Act queue
            with tc.high_priority(offset=40):
                if LH > 2:
                    nc.scalar.copy(nAs[:, 0:2, :], nA[:, 0:2, :])
                    nc.scalar.copy(nAs[:, 2:LH, :], nA[:, 2:LH, :])
                else:
                    nc.scalar.copy(nAs[:], nA[:])
            for l in range(LH):
                nc.tensor.matmul(
                    comb[S : 2 * S, l, :], nAs[:, l, :], identf[:],
                    tile_position=(0, 64),
                )
            v[f"comb{lh}"] = comb

        def pmx_scan(v, u, lh, pmxC):
            LH, l0 = PMX_SIZES[lh], PMX_OFF[lh]
            lsl = slice(l0, l0 + LH)
            comb = v.pop(f"comb{lh}")
            out_v = pmxC[:, lsl][:, :, None].broadcast_to([128, LH, S])
            mul_maxscan_seg(nc, out_v, comb[:], v["RAB"][:, lsl, :])

        def phaseB(b, d, ramp=False):
            """Staging DMAs + broadcasts, one pipeline stage after phaseA:
            every input here was computed a full period ago, so these DMAs
            never block the SP sequencer on data waits.  At ramp only the
            chunk0-critical staging + attP/attH go out now; the rest is
            emitted after phaseS(0) so it cannot hog the DMA wire first."""
            v = st[(b, d)]
            att_dram = dram_pool.tile([2, S, S], BF16, tag="att_d")
            nc.sync.dma_start(
                att_dram[:].rearrange("d s q -> (d s) q"), v["GsPK"][:]
            )
            attP = ab_pool.tile([128, S, S], BF16, tag="attP")
            attH = ab_pool.tile([128, S, S], BF16, tag="attH")
            bcast_rows(attP[:], att_dram[0].rearrange("s q -> (s q)"), 128)
            bcast_rows(attH[:], att_dram[1].rearrange("s q -> (s q)"), 128)
            v.update(attP=attP, attH=attH, att_dram=att_dram)
            if not ramp:
                phaseB_rest(b, d)

        def phaseB_rest(b, d):
            v = st[(b, d)]
            u = 2 * b + d
            att_dram = v.pop("att_dram")
            att1u = lg_pool.tile([128, S, S], BF16, tag="att1u", name=f"att1u{u}")
            bcast_sides(att1u[:], att_dram[:], S * S, S)
            # chunk2 (rows 192..200): stage the 8 v2T rows of each side and
            # replicate across the scan lanes (side-packed like GsPK)
            vt2_dram = dram_pool.tile([2, 8, S], BF16, tag="vt2_d")
            nc.sync.dma_start(vt2_dram[0], v["HT"][S : S + 8, 1, :])
            nc.sync.dma_start(vt2_dram[1], v["PT"][S : S + 8, 1, :])
            rep2 = lg_pool.tile([128, 8, S], BF16, tag="rep2", name=f"rep2{u}")
            bcast_sides(rep2[:], vt2_dram[:], 8 * S, S)
            rnab_dram = dram_pool.tile([L, 2, S], BF16, tag="rnab_d")
            nc.sync.dma_start(rnab_dram[:], v["rnab_b"][:])
            RAB = rb_pool.tile([128, L, S], BF16, tag="RAB")
            bcast_rows(RAB[0:S], rnab_dram[:, 1, :], S)
            bcast_rows(RAB[S : 2 * S], rnab_dram[:, 0, :], S)
            v.update(att1u=att1u, rep2=rep2, RAB=RAB)

        def phaseS(b, d, ramp=False):
            v = st[(b, d)]
            PT, HT = v["PT"], v["HT"]
            attP, attH = v["attP"], v["attH"]
            w_pair = w2t[2 + d]
            u = 2 * b + d

            pmxC = sc_pool.tile([128, L], F32, tag="pmxC", name=f"pmxC{u}")

            # ---- max-attentive chunk0 scans (one op per side) ----
            MhT = t_pool.tile([128, 2, S], BF16, tag="MhT", name=f"MhT{u}")
            MpT = t_pool.tile([128, 2, S], BF16, tag="MpT", name=f"MpT{u}")
            for (VT, attB, MT) in ((HT, attP, MhT), (PT, attH, MpT)):
                vt_b = VT[:, 0, :][:, None, :].broadcast_to([128, S, S])
                out_v = MT[:, 0, :][:, :, None].broadcast_to([128, S, S])
                mul_maxscan_seg(nc, out_v, vt_b, attB[:, :, :])
            v["MhT"], v["MpT"] = MhT, MpT
            if ramp:
                # ramp: the non-chunk0 broadcasts were held back so they
                # could not precede attP/attH on the DMA wire; emit now
                phaseB_rest(b, d)
            # chunk1b scan: rows 128..192, both sides
            MT1u = lg_pool.tile([128, S], BF16, tag="MT1u", name=f"MT1u{u}")
            in0 = v["VT1u"][:, None, :].broadcast_to([128, S, S])
            out_v = MT1u[:, :, None].broadcast_to([128, S, S])
            mul_maxscan_seg(nc, out_v, in0, v["att1u"][:])
            v["MT1u"] = MT1u
            # chunk2 page-dim scan: rows 192..200, both sides in one op
            # (lanes 0..64 = p with Gs rows, 64..128 = q with G2T rows)
            rep2 = v["rep2"]
            MT2s = lg_pool.tile([128, 8], BF16, tag="MT2s", name=f"MT2s{u}")
            in0 = v["GsPK"][:, None, :].broadcast_to([128, 8, S])
            out_v = MT2s[:, :, None].broadcast_to([128, 8, S])
            mul_maxscan_seg(nc, out_v, in0, rep2[:])
            v["MT2s"] = MT2s

            # pairwise passes: supply+scan interleaved in execution order —
            # pass p's supply runs on PE/Pool during the scans ahead of it
            # (ring-2 comb: pass p reuses pass p-2's buffer)
            for lh in range(len(PMX_SIZES)):
                pmx_supply(v, u, lh)
                pmx_scan(v, u, lh, pmxC)
            v["pmxC"] = pmxC

        def phaseT(b, d):
            stg_p, stg_h = stgs[b]
            v = st[(b, d)]
            PT, HT, PT2, HT2 = v["PT"], v["HT"], v["PT2"], v["HT2"]
            MS_hT = v["MS_T"][:, 0:2, :]
            MS_pT = v["MS_T"][:, 2:4, :]
            rnaT = v["rnT"][:, 0, :]
            MhT, MpT = v["MhT"], v["MpT"]
            pmxC = v["pmxC"]
            w_full = w2t[0 + d]
            w_mean = w2t[4 + d]
            w_amax = w2t[6 + d]
            so = 4 * d
            u = 2 * b + d

            # rows 128..192 from the per-unit packed group
            MT1u = v["MT1u"]
            nc.scalar.copy(MhT[0:S, 1, :], MT1u[0:S])
            # phase-T-local PSUM home: keeps phaseT out of the phaseA
            # psA/mmA rings so the deeper pipeline never WAR-stalls
            psT = ps_T.tile([128, S], F32, tag="psT", name=f"psT{u}")
            nc.tensor.matmul(psT[0:S, :], ident2[S : 2 * S, :],
                             MT1u[S : 2 * S], start=True, stop=True)
            nc.scalar.copy(MpT[0:S, 1, :], psT[0:S, :])
            # rows 192..200: one [128,8] -> [8,128] transpose covers both
            # sides (cols 0..64 = Mh rows, 64..128 = Mp rows)
            MT2s = v["MT2s"]
            c2t = psT[S : S + 8, :].bitcast(BF16)
            nc.tensor.transpose(c2t, MT2s[:, :], ident128[:])
            nc.scalar.copy(MhT[S : S + 8, 1, :], c2t[:, 0:S])
            nc.scalar.copy(MpT[S : S + 8, 1, :], c2t[:, S : 2 * S])

            mvhmax = sc_pool.tile([128, L], F32, tag="mvhmax", name=f"mvhm{u}")
            nc.gpsimd.tensor_tensor(
                mvhmax[S : 2 * S], pmxC[S : 2 * S],
                v["rnT"][S : 2 * S, 1, :], op=mybir.AluOpType.mult,
            )
            mvhmax_tiles[(b, d)] = mvhmax

            # ---- mp_match cosine accumulators: one PSUM bank ----
            blk = ps_blk.tile([S, 6, 4, L], F32, tag="blk")
            DOTp, NAp, NBp = blk[:, 0], blk[:, 1], blk[:, 2]
            DOTh, NAh, NBh = blk[:, 3], blk[:, 4], blk[:, 5]

            def mm_to(dst_ap, streams):
                for c, (c0, cs) in enumerate(CHUNKS):
                    nc.tensor.matmul(
                        dst_ap, streams[0](c), streams[1](c),
                        start=(c == 0), stop=(c == 1),
                    )

            def mp_stream(DOT, NA, NB, slot, v1T2, zT, z2T, w):
                mm_to(DOT[:, slot, :], (lambda c: zT[: CHUNKS[c][1], c, :],
                                        lambda c: w[: CHUNKS[c][1], c, :]))
                mm_to(NB[:, slot, :], (lambda c: z2T[: CHUNKS[c][1], c, :],
                                       lambda c: w[: CHUNKS[c][1], c, :]))

            def mp_na(NA, v1T2, ws):
                # NA = sum_h w^2 v1^2 for the three slots; per-slot
                # start/stop groups kept contiguous (interleaving
                # accumulation groups corrupts PSUM on HW)
                for slot, w in ws:
                    for c, (c0, cs) in enumerate(CHUNKS):
                        nc.tensor.matmul(
                            NA[:, slot, :], v1T2[:cs, c, :], w[:cs, c, :],
                            start=(c == 0), stop=(c == 1),
                        )

            def mk_z(v1T, v2T, tag, split=False):
                zT = t_pool.tile([128, 2, S], BF16, tag=f"zT_{tag}")
                z2T = t_pool.tile([128, 2, S], BF16, tag=f"z2T_{tag}")
                if split:
                    # per-chunk ops so chunk-0 streams start before the
                    # chunk-1 repack lands
                    for c in range(2):
                        nc.gpsimd.tensor_tensor(
                            zT[:, c, :], v1T[:, c, :], v2T[:, c, :],
                            op=mybir.AluOpType.mult,
                        )
                        nc.gpsimd.tensor_tensor(
                            z2T[:, c, :], v2T[:, c, :], v2T[:, c, :],
                            op=mybir.AluOpType.mult,
                        )
                else:
                    nc.gpsimd.tensor_tensor(
                        zT[:], v1T[:, :, :], v2T[:, :, :],
                        op=mybir.AluOpType.mult,
                    )
                    nc.gpsimd.tensor_tensor(
                        z2T[:], v2T[:, :, :], v2T[:, :, :],
                        op=mybir.AluOpType.mult,
                    )
                return zT, z2T

            class _ColBcast:
                def __init__(self, col2):
                    self.col2 = col2

                def __getitem__(self, idx):
                    sl, c, _ = idx
                    return self.col2[sl, c : c + 1].broadcast_to(
                        [self.col2[sl].shape[0], S]
                    )

            def mp_full(DOT, NA, NB, v1T, v1T2, vT, w, tag, j):
                zT = t_pool.tile([128, 2, S], BF16, tag=f"zT_f{tag}")
                col2 = sc_pool.tile([128, 2], BF16, tag=f"col2_{tag}")
                colf = sc_pool.tile([128, 2], F32, tag=f"colf_{tag}")
                nc.scalar.copy(colf[:], vT[:, :, j])
                for c, (c0, cs) in enumerate(CHUNKS):
                    nc.scalar.mul(
                        zT[:cs, c, :], v1T[:cs, c, :], colf[:cs, c : c + 1]
                    )
                nc.scalar.square(col2[:], vT[:, :, j])
                mp_stream(DOT, NA, NB, SLOT_FULL, v1T2, zT, _ColBcast(col2), w)

            j = 63 if d == 0 else 0
            ws3 = [(SLOT_AMAX, w_amax), (SLOT_FULL, w_full), (SLOT_MEAN, w_mean)]
            mp_na(NAp, PT2, ws3)
            mp_na(NAh, HT2, ws3)
            z, z2 = mk_z(PT, MhT, "pamax", split=True)
            mp_stream(DOTp, NAp, NBp, SLOT_AMAX, PT2, z, z2, w_amax)
            z, z2 = mk_z(HT, MpT, "hamax", split=True)
            mp_stream(DOTh, NAh, NBh, SLOT_AMAX, HT2, z, z2, w_amax)
            mp_full(DOTp, NAp, NBp, PT, PT2, HT, w_full, "p", j)
            mp_full(DOTh, NAh, NBh, HT, HT2, PT, w_full, "h", j)
            z, z2 = mk_z(PT, MS_hT, "pmean")
            mp_stream(DOTp, NAp, NBp, SLOT_MEAN, PT2, z, z2, w_mean)
            z, z2 = mk_z(HT, MS_pT, "hmean")
            mp_stream(DOTh, NAh, NBh, SLOT_MEAN, HT2, z, z2, w_mean)

            # ---- cosine tails: both sides batched per instruction ----
            nab2 = sc_pool.tile([S, 6, 4, L], F32, tag="nab2")
            nc.scalar.copy(nab2[:], blk[:])
            prod = sc_pool.tile([S, 8, L], F32, tag="prod")
            nc.gpsimd.tensor_tensor(
                prod[:].rearrange("s (a b) l -> s a b l", a=2),
                nab2[:, 1::3], nab2[:, 2::3], op=mybir.AluOpType.mult,
            )
            r = sc_pool.tile([S, 8, L], F32, tag="r")
            rsqrt(nc, sc_pool, r[:], prod[:], "dd")
            for side, stg in ((0, stg_p), (1, stg_h)):
                nc.gpsimd.tensor_tensor(
                    stg[:, so : so + 4, :], nab2[:, 3 * side],
                    r[:, 4 * side : 4 * side + 4, :],
                    op=mybir.AluOpType.mult,
                )
            nc.gpsimd.tensor_tensor(
                stg_p[:, so + SLOT_MAX, :], pmxC[0:S], rnaT[0:S],
                op=mybir.AluOpType.mult,
            )

        def emit_outputs(b):
            stg_p, stg_h = stgs[b]
            nc.sync.dma_start(mvp[b], stg_p[:].rearrange("s k l -> s (k l)"))
            # the over-p max rows are ready before the cosine tails; write the
            # rest of mvh around their columns so nothing serializes on them
            for d in range(2):
                so = 4 * d
                nc.sync.dma_start(
                    mvh[b, :, (so + SLOT_MAX) * L : (so + SLOT_MAX + 1) * L],
                    mvhmax_tiles[(b, d)][S : 2 * S],
                )
            for c0, c1 in ((0, SLOT_MAX), (SLOT_MAX + 1, 4 + SLOT_MAX), (4 + SLOT_MAX + 1, 8)):
                nc.sync.dma_start(
                    mvh[b, :, c0 * L : c1 * L],
                    stg_h[:, c0:c1, :].rearrange("s k l -> s (k l)"),
                )

        units = [(b, d) for b in range(BPC) for d in range(2)]
        NU = len(units)
        build_w2t(2)
        build_w2t(3)
        loadA(*units[0])
        loadA(*units[1])
        loadA(*units[2])
        phaseA(*units[0])
        for i in (0, 1, 4, 5, 6, 7):
            build_w2t(i)
        phaseA(*units[1])
        phaseB(units[0][0], units[0][1], ramp=True)
        # steady state: load(k+3) | A(k+2) | B(k+1) | S(k) | T(k-1).
        # S/T are EMITTED first: the scheduler's priority heap follows
        # emission order per queue, so the ready scans of S(k) must sit
        # ahead of A(k+2)'s long-latency chains (recips etc.) on DVE.
        for k in range(NU):
            phaseS(units[k][0], units[k][1], ramp=(k == 0))
            if k >= 1:
                phaseT(*units[k - 1])
            if k + 3 < NU:
                loadA(*units[k + 3])  # depth-3 lookahead
            if k + 2 < NU:
                phaseA(*units[k + 2])
            if k + 1 < NU:
                phaseB(*units[k + 1])
            # emit one stage after the pair's phaseT so the output DMAs
            # never hold the SP sequencer waiting on the cosine tails
            if k >= 2 and units[k - 2][1] == 1:
                emit_outputs(units[k - 2][0])
        phaseT(*units[-1])
        emit_outputs(units[-1][0])


_CACHE = {}


def _get_program():
    if "nc" not in _CACHE:
        _CACHE["nc"] = _build_core_program()
    return _CACHE["nc"]


def _run(inputs, trace=False):
    nc = _get_program()
    con_p = np.ascontiguousarray(np.asarray(inputs["con_p"], dtype=np.float32))
    con_h = np.ascontiguousarray(np.asarray(inputs["con_h"], dtype=np.float32))
    in_maps = []
    for c in range(NCORES):
        m = {
            "cp": con_p[c * BPC : (c + 1) * BPC],
            "ch": con_h[c * BPC : (c + 1) * BPC],
        }
        for i in range(8):
            m[f"w{i+1}"] = np.ascontiguousarray(
                np.asarray(inputs[f"mp_w{i+1}"], dtype=np.float32)
            )
        in_maps.append(m)
    res = bass_utils.run_bass_kernel_spmd(
        nc, in_maps, core_ids=list(range(NCORES)), trace=trace
    )
    mv_p = np.concatenate([r["mvp"] for r in res.results], axis=0)
    mv_h = np.concatenate([r["mvh"] for r in res.results], axis=0)
    return (mv_p, mv_h), res


def kernel(**inputs):
    out, _ = _run(inputs, trace=False)
    return out


def predicted_ns():
    """Cost-model (TimelineSim) predicted on-device duration in ns."""
    try:
        from trails.perfetto import LazyPerfetto

        def _noop(self, *a, **k):
            return 0

        for m in (
            "enable_explicit_ordering",
            "reserve_process_order",
            "add_counter",
        ):
            if not hasattr(LazyPerfetto, m):
                setattr(LazyPerfetto, m, _noop)
        from concourse.timeline_sim import TimelineSim

        return float(TimelineSim(_get_program(), trace=False).simulate())
    except Exception:
        return None


def run_profiled(**inputs):
    """Run on hardware; return (outputs, exec_time_ns).

    NTFF hardware tracing is unavailable under this axon client (no
    antenv.axon_hooks), so exec time falls back to the cost-model
    prediction, which the optimization loop was driven by.
    """
    try:
        out, res = _run(inputs, trace=True)
        ns = res.exec_time_ns
    except ModuleNotFoundError:
        out, _ = _run(inputs, trace=False)
        ns = None
    if ns is None:
        ns = predicted_ns()
    return out, ns

